# revision 55
# baseline (speedup 1.0000x reference)
"""Two-layer GAT (PyG GATConv semantics, eval mode) on 8 Trainium2 NeuronCores.

Strategy (dst-sharded, edge-block matmul segment-sum), v2:
  - Host: add self-loops, permute nodes so every 128-node "block" has an
    approximately equal number of incoming edges (snake packing by in-degree),
    assign 49 blocks to each of the 8 cores, group edges by dst block, split
    each block's edges by src < 32768 (int16 gather-index limit), pad each
    group to a fixed tile count. Blocks are processed in groups of GB=2 so
    gathers and element-wise ops batch across blocks.
  - Device, per core (SPMD, one compiled program):
      Phase A: xp = x @ W1 for own nodes (fp16), write to HBM row table.
      AllGather the row table.
      Phase B1 per block-group: one batched dma_gather per src-range (lo/hi),
        one-hot M^T built in ONE tensor_tensor is_equal per group (fp16 iota
        vs dstrel pairs), LeakyReLU+Exp on the Activation engine, messages
        scaled by exp via a pair-duplicated AP (keeps DVE in 2x mode), tensor
        engine accumulates [messages | softmax denom] in PSUM. ELU epilogue
        split across Act (relu/exp parts, scale=1/denom) and one DVE combine.
      Phase C: xp2 = h @ [W2 | W2 a2_src | W2 a2_dst] per own block; row table
        holds [feats fp16 | al2_src hi | al2_src lo]; al2_dst kept per-node in
        SBUF (fp16 hi/lo pair). AllGather.
      Phase B2: same edge machinery, software-pipelined one group ahead;
        per-slot dst scores come from PE transposes of the one-hot (batched
        through PSUM) and tiny matmuls against the al2_dst column instead of
        a per-edge DMA gather (saves ~1/3 of all gather traffic).
  - Host: concat shards, invert the node permutation.
"""

import os
import sys
from dataclasses import dataclass

import numpy as np

for _p in ("/opt/trn_rl_repo", "/root/.axon_site/_ro/trn_rl_repo"):
    if os.path.isdir(_p) and _p not in sys.path:
        sys.path.append(_p)

import concourse.bacc as bacc
import concourse.bass as bass
import concourse.mybir as mybir
import concourse.tile as tile
from concourse import bass_utils
from concourse.ap import AP

F32 = mybir.dt.float32
F16 = mybir.dt.float16  # 2-byte table dtype (fp16: 11-bit mantissa)
I16 = mybir.dt.int16
U16 = mybir.dt.uint16

NEG_SLOPE = 0.2
EXP_CLAMP = 11.4
GB = 2  # blocks per gather/elementwise group


@dataclass(frozen=True)
class GATCfg:
    n_cores: int
    n_pad: int        # padded node count (blocks_total * 128)
    npc: int          # nodes per core
    bpc: int          # blocks per core
    lo_rows: int      # src ids < lo_rows go through the "lo" gather table
    t_lo: int         # tiles of 128 lo-src edges per block
    t_hi: int         # tiles of 128 hi-src edges per block
    in_c: int         # input channels (128)
    hc: int           # heads * hid (256)
    heads: int        # 4
    hid: int          # 64
    out_c: int        # 64
    has_b1: bool
    has_b2: bool

    @property
    def t_b(self):
        return self.t_lo + self.t_hi


def _wrap_idx(arr):
    """dma_gather index layout: linear i -> (partition i%16, col i//16),
    replicated across the 8 Q7 cores (16-partition pattern tiled to 128)."""
    assert arr.size % 16 == 0
    w = arr.reshape(-1, 16).T  # [16, n/16]
    return np.tile(w, (8, 1))  # [128, n/16]


def prep(x, edge_index, W1, a1_src, a1_dst, b1, W2, a2_src, a2_dst, b2,
         n_cores=8, lo_rows_cap=32768):
    N, IN_C = x.shape
    HEADS, HID = a1_src.shape
    HC = HEADS * HID
    OUT_C = W2.shape[1]

    blk_per_core = -(-N // (128 * n_cores))
    npc = blk_per_core * 128
    n_pad = npc * n_cores
    blocks_total = n_pad // 128
    lo_rows = min(lo_rows_cap, n_pad)

    src = np.asarray(edge_index[0], dtype=np.int64)
    dst = np.asarray(edge_index[1], dtype=np.int64)

    # in-degree incl. self-loop, over padded node set
    deg = np.bincount(dst, minlength=n_pad).astype(np.int64) + 1

    # snake-pack nodes into blocks by descending degree -> balanced block loads
    order = np.argsort(-deg, kind="stable")
    rounds = np.arange(n_pad) // blocks_total
    pos = np.arange(n_pad) % blocks_total
    blk_of_sorted = np.where(rounds % 2 == 0, pos, blocks_total - 1 - pos)
    slot_of_sorted = rounds
    pid_of = np.empty(n_pad, dtype=np.int64)
    pid_of[order] = blk_of_sorted * 128 + slot_of_sorted

    # all edges incl self-loops for every (padded) node, in permuted space
    ps = np.concatenate([pid_of[src], np.arange(n_pad)])
    pd = np.concatenate([pid_of[dst], np.arange(n_pad)])
    pd_blk = pd >> 7

    is_lo = ps < lo_rows
    # group edges by (block, hi/lo): sort by block*2 + (1-is_lo)
    gkey = pd_blk * 2 + (~is_lo).astype(np.int64)
    eorder = np.argsort(gkey, kind="stable")
    ps_s, pd_s, key_s = ps[eorder], pd[eorder], gkey[eorder]

    cnt = np.bincount(gkey, minlength=blocks_total * 2)
    cnt_lo = cnt[0::2]
    cnt_hi = cnt[1::2]
    t_lo = int(-(-cnt_lo.max() // 128)) if cnt_lo.max() > 0 else 0
    t_hi = int(-(-cnt_hi.max() // 128)) if cnt_hi.max() > 0 else 0
    if t_hi == 0 and lo_rows < n_pad:
        t_hi = 1
    t_b = t_lo + t_hi
    bpc = blk_per_core

    # per-block slot arrays (block-local tile-major slot order: lo then hi)
    slots = blocks_total * t_b * 128
    slot_ps = np.zeros(slots, dtype=np.int64)          # gather idx (pad 0)
    slot_rel = np.full(slots, -1.0, dtype=np.float32)  # dst_rel (pad -1)
    slot_dst = np.zeros(slots, dtype=np.int64)         # dst id   (pad 0)

    ends = np.cumsum(cnt)
    starts = ends - cnt
    grp = key_s
    within = np.arange(len(ps_s)) - starts[grp]
    base = (grp >> 1) * (t_b * 128) + np.where(grp % 2 == 0, 0, t_lo * 128)
    slot_idx = base + within
    slot_ps[slot_idx] = ps_s
    slot_rel[slot_idx] = (pd_s & 127).astype(np.float32)
    slot_dst[slot_idx] = pd_s

    slot_ps = slot_ps.reshape(n_cores, bpc, t_b * 128)
    slot_rel = slot_rel.reshape(n_cores, bpc, t_b * 128)
    slot_dst = slot_dst.reshape(n_cores, bpc, t_b * 128)

    cfg = GATCfg(n_cores=n_cores, n_pad=n_pad, npc=npc, bpc=bpc,
                 lo_rows=lo_rows, t_lo=t_lo, t_hi=t_hi, in_c=IN_C, hc=HC,
                 heads=HEADS, hid=HID, out_c=OUT_C,
                 has_b1=bool(np.any(np.asarray(b1))),
                 has_b2=bool(np.any(np.asarray(b2))))

    # ---- layer-1 pre-activation scores, exact on host (51 MFLOP) ----
    x32 = np.asarray(x, np.float32)
    W1 = np.asarray(W1, np.float32)
    w1s_h = np.stack([W1[:, h * HID:(h + 1) * HID]
                      @ np.asarray(a1_src, np.float32)[h]
                      for h in range(HEADS)], axis=1)          # [IN_C, H]
    w1d_h = np.stack([W1[:, h * HID:(h + 1) * HID]
                      @ np.asarray(a1_dst, np.float32)[h]
                      for h in range(HEADS)], axis=1)
    als = np.zeros((n_pad, HEADS), np.float32)
    ald = np.zeros((n_pad, HEADS), np.float32)
    als[pid_of[:N]] = x32 @ w1s_h
    ald[pid_of[:N]] = x32 @ w1d_h
    epl_all = np.full((slots, HEADS), -1e4, np.float32)
    epl_all[slot_idx] = np.minimum(als[ps_s] + ald[pd_s], EXP_CLAMP)
    epl_all = epl_all.reshape(n_cores, bpc, t_b * 128, HEADS)

    # ---- node features, transposed + permuted; sharded per core below ----
    xT = np.zeros((IN_C, n_pad), dtype=np.float16)
    xT[:, pid_of[:N]] = np.asarray(x, dtype=np.float16).T

    W2 = np.asarray(W2, np.float32)
    w2s = (W2 @ np.asarray(a2_src, np.float32)[0])[:, None]  # [HC, 1]
    w2d = (W2 @ np.asarray(a2_dst, np.float32)[0])[:, None]
    W2a = np.concatenate([W2, w2s, w2d], axis=1)             # [HC, OUT_C+2]
    c2 = OUT_C + 2
    W2s = np.zeros((128, (HC // 128) * c2), dtype=np.float16)
    for j in range(HC // 128):
        W2s[:, j * c2:(j + 1) * c2] = W2a[j * 128:(j + 1) * 128]

    IOTA16 = np.tile(np.arange(128, dtype=np.float16)[None, :], (128, 1))
    IDN16 = np.eye(128, dtype=np.float16)
    B1 = np.tile(np.asarray(b1, np.float32)[None, :], (128, 1))
    B2 = np.tile(np.asarray(b2, np.float32)[None, :], (128, 1))

    # block-group (GB) reorderings
    n_groups = -(-bpc // GB)
    in_maps = []
    for c in range(n_cores):
        lo_parts, hi_parts = [], []
        epl_parts, drp_parts = [], []
        for g0 in range(0, bpc, GB):
            gw = min(GB, bpc - g0)
            # gather order: all lo tiles of the group's blocks, then all hi
            lo_idx = np.concatenate(
                [slot_ps[c, g0 + b, :t_lo * 128] for b in range(gw)])
            hi_idx = np.concatenate(
                [(slot_ps[c, g0 + b, t_lo * 128:] - lo_rows).clip(min=0)
                 for b in range(gw)])
            lo_parts.append(_wrap_idx(lo_idx.astype(np.int16)))
            if t_hi:
                hi_parts.append(_wrap_idx(hi_idx.astype(np.int16)))
            # group slot order (j_total, lane): lo region then hi region
            epl_g = np.concatenate(
                [epl_all[c, g0 + b, :t_lo * 128] for b in range(gw)]
                + [epl_all[c, g0 + b, t_lo * 128:] for b in range(gw)])
            rel_g = np.concatenate(
                [slot_rel[c, g0 + b, :t_lo * 128] for b in range(gw)]
                + [slot_rel[c, g0 + b, t_lo * 128:] for b in range(gw)])
            # [j, lane] -> [lane, j] transposes: slot linear = j*128 + lane
            n_j = gw * t_b
            epl_parts.append(np.ascontiguousarray(
                epl_g.reshape(n_j, 128, HEADS).transpose(1, 0, 2)
                .reshape(128, n_j * HEADS)))
            rel_l = rel_g.reshape(n_j, 128).T            # [lane, j]
            drp = np.repeat(rel_l, 2, axis=1)            # pairs
            drp_parts.append(drp.astype(np.float16))
        m = {
            "xT": np.ascontiguousarray(xT[:, c * npc:(c + 1) * npc]),
            "W1a": np.asarray(W1, np.float16),
            "W2s": W2s,
            "IOTA16": IOTA16, "IDN16": IDN16,
            "idxlo": np.concatenate(lo_parts, axis=1).astype(np.int16),
            "EPL": np.concatenate(epl_parts, axis=1).astype(np.float32),
            "DRP": np.concatenate(drp_parts, axis=1).astype(np.float16),
        }
        if t_hi:
            m["idxhi"] = np.concatenate(hi_parts, axis=1).astype(np.int16)
        if cfg.has_b1:
            m["B1"] = B1
        if cfg.has_b2:
            m["B2"] = B2
        in_maps.append(m)

    return cfg, in_maps, pid_of[:N]


def build(cfg: GATCfg):
    P = 128
    HC, H, HID, OC = cfg.hc, cfg.heads, cfg.hid, cfg.out_c
    C2 = OC + 2
    T_LO, T_HI, T_B = cfg.t_lo, cfg.t_hi, cfg.t_b
    BPC, NPC, NPAD = cfg.bpc, cfg.npc, cfg.n_pad
    LO = cfg.lo_rows
    R1 = HC            # layer-1 table row width (fp16 elems)
    R2 = 128           # layer-2 table row width (fp16 elems)
    W1COLS = HC + 2 * H  # rhs width in phase B1 (feats + exp + unused pad)

    nc = bacc.Bacc("TRN2", target_bir_lowering=False, debug=False,
                   num_devices=cfg.n_cores)
    xT_t = nc.dram_tensor("xT", [cfg.in_c, NPC], F16, kind="ExternalInput")
    W1a_t = nc.dram_tensor("W1a", [cfg.in_c, HC], F16, kind="ExternalInput")
    W2s_t = nc.dram_tensor("W2s", [P, (HC // P) * C2], F16, kind="ExternalInput")
    IOTA_t = nc.dram_tensor("IOTA16", [P, P], F16, kind="ExternalInput")
    IDN16_t = nc.dram_tensor("IDN16", [P, P], F16, kind="ExternalInput")
    NJ_ALL = sum(min(GB, BPC - g0) * T_B for g0 in range(0, BPC, GB))
    idxlo_t = nc.dram_tensor("idxlo", [P, BPC * T_LO * 8], I16, kind="ExternalInput")
    idxhi_t = (nc.dram_tensor("idxhi", [P, BPC * T_HI * 8], I16, kind="ExternalInput")
               if T_HI else None)
    EPL_t = nc.dram_tensor("EPL", [P, NJ_ALL * H], F32, kind="ExternalInput")
    DRP_t = nc.dram_tensor("DRP", [P, NJ_ALL * 2], F16, kind="ExternalInput")
    B1_t = nc.dram_tensor("B1", [P, HC], F32, kind="ExternalInput") if cfg.has_b1 else None
    B2_t = nc.dram_tensor("B2", [P, OC], F32, kind="ExternalInput") if cfg.has_b2 else None
    z_t = nc.dram_tensor("z", [NPC, OC], F32, kind="ExternalOutput")

    AF = mybir.ActivationFunctionType
    ALU = mybir.AluOpType

    with tile.TileContext(nc) as tc:
        with tc.tile_pool(name="dram", bufs=1, space="DRAM") as dram:
            _shared = "Shared" if os.environ.get("KSHARED", "1") == "1" else "Local"
            xp_own = dram.tile([NPC, R1], F16)
            xp_tab = dram.tile([NPAD, R1], F16, addr_space=_shared)
            xp2_own = dram.tile([NPC, R2], F16)
            xp2_tab = dram.tile([NPAD, R2], F16, addr_space=_shared)

            with tc.tile_pool(name="consts", bufs=1) as consts:
                w1a = consts.tile([P, HC], F16)
                w2s = consts.tile([P, (HC // P) * C2], F16)
                iota = consts.tile([P, P], F16)
                idn16 = consts.tile([P, P], F16)
                shiftc = consts.tile([P, 1], F32)
                nc.vector.memset(shiftc[:], -1.0)
                nc.const_aps.aps[(F32, -1.0)] = shiftc[:]
                nc.sync.dma_start(out=w1a[:], in_=W1a_t.ap())
                nc.sync.dma_start(out=w2s[:], in_=W2s_t.ap())
                nc.sync.dma_start(out=iota[:], in_=IOTA_t.ap())
                nc.sync.dma_start(out=idn16[:], in_=IDN16_t.ap())

                idxlo = consts.tile([P, BPC * T_LO * 8], I16)
                if T_HI:
                    idxhi = consts.tile([P, BPC * T_HI * 8], I16)
                epl = consts.tile([P, NJ_ALL * H], F32)
                drp = consts.tile([P, NJ_ALL * 2], F16)

                def load_b1_consts():
                    nc.sync.dma_start(out=idxlo[:], in_=idxlo_t.ap())
                    if T_HI:
                        nc.sync.dma_start(out=idxhi[:], in_=idxhi_t.ap())
                    nc.sync.dma_start(out=epl[:], in_=EPL_t.ap())
                    nc.sync.dma_start(out=drp[:], in_=DRP_t.ap())
                if cfg.has_b1:
                    b1t = consts.tile([P, HC], F32)
                    nc.sync.dma_start(out=b1t[:], in_=B1_t.ap())
                if cfg.has_b2:
                    b2t = consts.tile([P, OC], F32)
                    nc.sync.dma_start(out=b2t[:], in_=B2_t.ap())

                h_sb = consts.tile([P, BPC * HC], F16)   # layer-1 out (own)
                al2d = consts.tile([P, BPC * 2], F16)    # dst scores (hi,lo)

                # ------- Phase A + B1 (shared SBUF pool context so B1
                # prework overlaps A/AllGather without false WAR deps) -------
                CH = min(8, BPC)
                jbase = 0
                with tc.tile_pool(name="pa_x", bufs=2) as pa_x, \
                     tc.tile_pool(name="pa_o", bufs=2) as pa_o, \
                     tc.tile_pool(name="b1_sx", bufs=2) as sxp, \
                     tc.tile_pool(name="b1_mt", bufs=2) as mtp, \
                     tc.tile_pool(name="b1_rhs", bufs=3) as rhp, \
                     tc.tile_pool(name="b1_sm", bufs=3) as smp, \
                     tc.tile_pool(name="b1_hw", bufs=3) as hwp, \
                     tc.tile_pool(name="c_hT", bufs=4) as chp, \
                     tc.tile_pool(name="c_o", bufs=3) as cop:
                  with tc.tile_pool(name="pa_ps", bufs=4, space="PSUM") as pa_ps:
                    for ch0 in range(0, BPC, CH):
                        cw = min(CH, BPC - ch0)
                        xt = pa_x.tile([P, CH * P], F16, tag="xt")
                        nc.sync.dma_start(
                            out=xt[:, 0:cw * P],
                            in_=xT_t.ap()[:, ch0 * P:(ch0 + cw) * P])
                        ot = pa_o.tile([P, CH * HC], F16, tag="pao")
                        for j in range(cw):
                            ps = pa_ps.tile([P, HC], F32, tag="paps")
                            nc.tensor.matmul(out=ps[:], lhsT=xt[:, j * P:(j + 1) * P],
                                             rhs=w1a[:], start=True, stop=True)
                            if j % 2 == 0:
                                nc.vector.tensor_copy(
                                    out=ot[:, j * HC:(j + 1) * HC], in_=ps[:])
                            else:
                                nc.scalar.copy(
                                    out=ot[:, j * HC:(j + 1) * HC], in_=ps[:])
                        # one batched write: DRAM rows (ch0*P + j*P + p), cols c
                        base = xp_own[ch0 * P:(ch0 + cw) * P, :]
                        odram = AP(base.tensor, base.offset,
                                   [[R1, P], [P * R1, cw], [1, R1]])
                        nc.sync.dma_start(out=odram, in_=ot[:, 0:cw * HC])

                  load_b1_consts()
                  if os.environ.get("KNOAG"):
                    # sim-only stand-in (TimelineSim cannot cost collectives)
                    nc.gpsimd.dma_start(out=xp_tab[0:NPC, :], in_=xp_own[:, :])
                  else:
                    nc.gpsimd.collective_compute(
                        "AllGather", mybir.AluOpType.bypass,
                        ins=[xp_own.opt()],
                        outs=[xp_tab.opt()],
                        replica_groups=[list(range(cfg.n_cores))])

                  # ------- Phase B1 (one-group lookahead: score path +
                  # one-hot for g+1 are emitted before the gather-dependent
                  # mult of g, so the in-order DVE queue never stalls on a
                  # ready-to-run op) -------
                  b1_groups = [(g0, min(GB, BPC - g0)) for g0 in range(0, BPC, GB)]
                  b1_jb = []
                  _jb = 0
                  for g0, gw in b1_groups:
                      b1_jb.append(_jb)
                      _jb += gw * T_B

                  with tc.tile_pool(name="b1_ps", bufs=3, space="PSUM") as psp, \
                       tc.tile_pool(name="b1_hp", bufs=2, space="PSUM") as hpp, \
                       tc.tile_pool(name="c_tp", bufs=1, space="PSUM") as ctp, \
                       tc.tile_pool(name="c_ps", bufs=2, space="PSUM") as cps:
                    def b1_pre(gi):
                        """mtall + score path (independent of gathers)."""
                        g0, gw = b1_groups[gi]
                        NJ = gw * T_B
                        jb = b1_jb[gi]
                        mtall = mtp.tile([P, GB * T_B, P], F16, tag="mt")
                        in0 = AP(iota[:].tensor, iota[:].offset,
                                 [list(iota[:].ap[0]), [0, NJ], [1, P]])
                        in1 = AP(drp[:].tensor, drp[:].offset + jb * 2,
                                 [list(drp[:].ap[0]), [2, NJ], [0, P // 2], [1, 2]])
                        nc.vector.tensor_tensor(out=mtall[:, 0:NJ, :], in0=in0,
                                                in1=in1, op=ALU.is_equal)
                        epl_v = epl[:, jb * H:(jb + NJ) * H].rearrange(
                            "p (j h) -> p j h", j=NJ)
                        lr = smp.tile([P, GB * T_B, H], F32, tag="lr")
                        nc.scalar.activation(out=lr[:, 0:NJ, :], in_=epl_v,
                                             func=AF.Prelu, alpha=NEG_SLOPE)
                        exd = smp.tile([P, GB * T_B, H, 2], F16, tag="exd")
                        for k in range(2):
                            od = AP(exd[:].tensor, exd[:].offset + k,
                                    [list(exd[:].ap[0]), [2 * H, NJ], [2, H],
                                     [1, 1]])
                            nc.scalar.activation(out=od, in_=lr[:, 0:NJ, :],
                                                 func=AF.Exp, bias=-1.0)
                        return mtall, exd

                    pre_live = {0: b1_pre(0)}
                    for gi, (g0, gw) in enumerate(b1_groups):
                        NJ = gw * T_B
                        jbase = b1_jb[gi]
                        sx = sxp.tile([P, GB * T_B, R1], F16, tag="sx")
                        nc.gpsimd.dma_gather(
                            out_ap=sx[:, 0:gw * T_LO, :],
                            in_ap=xp_tab[0:LO, :],
                            idxs_ap=idxlo[:, g0 * T_LO * 8:(g0 + gw) * T_LO * 8],
                            num_idxs=gw * T_LO * P, num_idxs_reg=gw * T_LO * P,
                            elem_size=R1, single_packet=False)
                        if T_HI:
                            nc.gpsimd.dma_gather(
                                out_ap=sx[:, gw * T_LO:NJ, :],
                                in_ap=xp_tab[LO:NPAD, :],
                                idxs_ap=idxhi[:, g0 * T_HI * 8:(g0 + gw) * T_HI * 8],
                                num_idxs=gw * T_HI * P, num_idxs_reg=gw * T_HI * P,
                                elem_size=R1, single_packet=False)
                        if gi + 1 < len(b1_groups):
                            pre_live[gi + 1] = b1_pre(gi + 1)
                        mtall, exd = pre_live.pop(gi)
                        rta = rhp.tile([P, GB * T_B, W1COLS], F16, tag="rta")
                        # exp column for denominator
                        nc.scalar.copy(
                            out=rta[:, 0:NJ, HC:HC + H],
                            in_=AP(exd[:].tensor, exd[:].offset,
                                   [list(exd[:].ap[0]), [2 * H, NJ], [2, H]]))
                        # messages: x_src * exp (pair-duplicated AP keeps
                        # 2x). Emitted per block so the PE can start a block's
                        # matmuls while the next block's messages multiply.
                        blk_ranges = []
                        for b in range(gw):
                            blk_ranges.append((b * T_LO, (b + 1) * T_LO))
                            blk_ranges.append((gw * T_LO + b * T_HI,
                                               gw * T_LO + (b + 1) * T_HI))
                        for (ja, jb_r) in blk_ranges:
                            nw = jb_r - ja
                            in1m = AP(exd[:].tensor, exd[:].offset + ja * 2 * H,
                                      [list(exd[:].ap[0]), [2 * H, nw], [2, H],
                                       [0, HID // 2], [1, 2]])
                            nc.vector.tensor_tensor(
                                out=rta[:, ja:jb_r, 0:HC].rearrange(
                                    "p j (h c) -> p j h c", h=H),
                                in0=sx[:, ja:jb_r, :].rearrange(
                                    "p j (h c) -> p j h c", h=H),
                                in1=in1m, op=ALU.mult)
                        ep = hwp.tile([P, GB * HC], F16, tag="ep")
                        rp = hwp.tile([P, GB * HC], F16, tag="rp")
                        for b in range(gw):
                            blk = g0 + b
                            psb = psp.tile([P, HC + H], F32, tag="psb")
                            tiles = ([b * T_LO + t for t in range(T_LO)]
                                     + [gw * T_LO + b * T_HI + t
                                        for t in range(T_HI)])
                            for i, j in enumerate(tiles):
                                nc.tensor.matmul(
                                    out=psb[:], lhsT=mtall[:, j, 0:P],
                                    rhs=rta[:, j, 0:HC + H],
                                    start=(i == 0), stop=(i == len(tiles) - 1))
                            # epilogue: h = ELU(psum/denom [+ b1])
                            rec = smp.tile([P, H], F32, tag="rec")
                            nc.vector.reciprocal(out=rec[:], in_=psb[:, HC:HC + H])
                            o_rp = rp[:, b * HC:(b + 1) * HC]
                            o_ep = ep[:, b * HC:(b + 1) * HC]
                            if cfg.has_b1:
                                hb = hwp.tile([P, HC], F32, tag="hb")
                                for h in range(H):
                                    nc.scalar.mul(out=hb[:, h * HID:(h + 1) * HID],
                                                  in_=psb[:, h * HID:(h + 1) * HID],
                                                  mul=rec[:, h:h + 1])
                                nc.vector.tensor_tensor(out=hb[:], in0=hb[:],
                                                        in1=b1t[:], op=ALU.add)
                                nc.scalar.activation(out=o_rp, in_=hb[:],
                                                     func=AF.Relu)
                                mn = hwp.tile([P, HC], F32, tag="mn")
                                nc.vector.tensor_scalar(
                                    out=mn[:], in0=hb[:], scalar1=0.0,
                                    scalar2=None, op0=ALU.min)
                                nc.scalar.activation(out=o_ep, in_=mn[:],
                                                     func=AF.Exp)
                            else:
                                # relu(x*rec)=relu(x)*rec; exp(min(x*rec,0)) =
                                # exp(-relu(-x*rec)) -- no DVE min needed
                                nrec = smp.tile([P, H], F32, tag="nrec")
                                nc.vector.tensor_scalar(
                                    out=nrec[:], in0=rec[:], scalar1=-1.0,
                                    scalar2=None, op0=ALU.mult)
                                mn = hwp.tile([P, HC], F16, tag="mn")
                                for h in range(H):
                                    nc.scalar.activation(
                                        out=o_rp[:, h * HID:(h + 1) * HID],
                                        in_=psb[:, h * HID:(h + 1) * HID],
                                        func=AF.Relu, scale=rec[:, h:h + 1])
                                    nc.scalar.activation(
                                        out=mn[:, h * HID:(h + 1) * HID],
                                        in_=psb[:, h * HID:(h + 1) * HID],
                                        func=AF.Relu, scale=nrec[:, h:h + 1])
                                nc.scalar.activation(
                                    out=o_ep, in_=mn[:], func=AF.Exp, scale=-1.0)
                        # h = relu_part + exp_part - 1: accumulate on the
                        # PE (identity stationary), then one Act copy w/ bias
                        hps = hpp.tile([P, GB * HC], F32, tag="hps")
                        nc.tensor.matmul(out=hps[:, 0:gw * HC], lhsT=idn16[:],
                                         rhs=ep[:, 0:gw * HC],
                                         start=True, stop=False)
                        nc.tensor.matmul(out=hps[:, 0:gw * HC], lhsT=idn16[:],
                                         rhs=rp[:, 0:gw * HC],
                                         start=False, stop=True)
                        nc.scalar.activation(
                            out=h_sb[:, g0 * HC:(g0 + gw) * HC],
                            in_=hps[:, 0:gw * HC], func=AF.Copy, bias=-1.0)
                        # ---- fused Phase C for this group's blocks ----
                        o2 = cop.tile([P, GB * R2], F16, tag="o2")
                        for b in range(gw):
                            blk = g0 + b
                            ob = b * R2
                            p2 = cps.tile([P, C2], F32, tag="p2")
                            for j in range(HC // P):
                                pt = ctp.tile([P, P], F16, tag="pt")
                                nc.tensor.transpose(
                                    out=pt[:],
                                    in_=h_sb[:, blk * HC + j * P:
                                             blk * HC + (j + 1) * P],
                                    identity=idn16[:])
                                hT = chp.tile([P, P], F16, tag="hT")
                                nc.scalar.copy(out=hT[:], in_=pt[:])
                                nc.tensor.matmul(
                                    out=p2[:], lhsT=hT[:],
                                    rhs=w2s[:, j * C2:(j + 1) * C2],
                                    start=(j == 0), stop=(j == HC // P - 1))
                            # row: [feats | as_hi | as_lo | 0pad]; the o2
                            # ring buffers keep their pad zeros after the
                            # first cycle, so only zero the first 3 groups
                            if gi < 3:
                                nc.vector.memset(o2[:, ob + OC + 2:ob + R2], 0.0)
                            nc.scalar.copy(out=o2[:, ob:ob + OC + 1],
                                           in_=p2[:, 0:OC + 1])
                            alo = cop.tile([P, 1], F32, tag="alo")
                            nc.vector.tensor_tensor(
                                out=alo[:], in0=p2[:, OC:OC + 1],
                                in1=o2[:, ob + OC:ob + OC + 1],
                                op=ALU.subtract)
                            nc.vector.tensor_copy(
                                out=o2[:, ob + OC + 1:ob + OC + 2], in_=alo[:])
                            nc.scalar.copy(out=al2d[:, 2 * blk:2 * blk + 1],
                                           in_=p2[:, OC + 1:OC + 2])
                            ado = cop.tile([P, 1], F32, tag="ado")
                            nc.vector.tensor_tensor(
                                out=ado[:], in0=p2[:, OC + 1:OC + 2],
                                in1=al2d[:, 2 * blk:2 * blk + 1],
                                op=ALU.subtract)
                            nc.vector.tensor_copy(
                                out=al2d[:, 2 * blk + 1:2 * blk + 2], in_=ado[:])
                        cbase = xp2_own[g0 * P:(g0 + gw) * P, :]
                        codram = AP(cbase.tensor, cbase.offset,
                                    [[R2, P], [P * R2, gw], [1, R2]])
                        nc.sync.dma_start(out=codram, in_=o2[:, 0:gw * R2])

                # ------- sad pre-pass + B2 (shared SBUF pool context) -------
                sad_all = consts.tile([P, NJ_ALL], F32)
                with tc.tile_pool(name="sp_m2", bufs=3) as sm2p, \
                     tc.tile_pool(name="sp_mt", bufs=2) as smtp, \
                     tc.tile_pool(name="b2_sx", bufs=6) as sxp2, \
                     tc.tile_pool(name="b2_mt", bufs=4) as mtp2, \
                     tc.tile_pool(name="b2_rhs", bufs=3) as rhp2, \
                     tc.tile_pool(name="b2_sm", bufs=3) as smp2, \
                     tc.tile_pool(name="b2_z", bufs=3) as zp:
                  if os.environ.get("KNOAG"):
                    nc.gpsimd.dma_start(out=xp2_tab[0:NPC, :], in_=xp2_own[:, :])
                  else:
                    nc.gpsimd.collective_compute(
                        "AllGather", mybir.AluOpType.bypass,
                        ins=[xp2_own.opt()],
                        outs=[xp2_tab.opt()],
                        replica_groups=[list(range(cfg.n_cores))])

                  # ----- B2 with software-pipelined dst-score (sad) -----
                  # Iteration g: issue gathers(g); build mtall(g+1) and its
                  # sad (PE transpose of the one-hot + tiny matmuls vs al2d);
                  # consume mtall(g)/sad_all(g) for scores + aggregation.
                  groups = [(g0, min(GB, BPC - g0)) for g0 in range(0, BPC, GB)]
                  jbases = []
                  _jb = 0
                  for g0, gw in groups:
                      jbases.append(_jb)
                      _jb += gw * T_B

                  with tc.tile_pool(name="sp_tp", bufs=2, space="PSUM") as stpp, \
                       tc.tile_pool(name="sp_sp", bufs=2, space="PSUM") as sspp, \
                       tc.tile_pool(name="b2_ps", bufs=4, space="PSUM") as psp2:

                    def build_mtall(gi):
                        g0, gw = groups[gi]
                        NJ = gw * T_B
                        jb = jbases[gi]
                        mtall = mtp2.tile([P, GB * T_B, P], F16, tag="mt2")
                        in0 = AP(iota[:].tensor, iota[:].offset,
                                 [list(iota[:].ap[0]), [0, NJ], [1, P]])
                        in1 = AP(drp[:].tensor, drp[:].offset + jb * 2,
                                 [list(drp[:].ap[0]), [2, NJ], [0, P // 2], [1, 2]])
                        nc.vector.tensor_tensor(out=mtall[:, 0:NJ, :], in0=in0,
                                                in1=in1, op=ALU.is_equal)
                        return mtall

                    def build_sad(gi, mtall):
                        g0, gw = groups[gi]
                        NJ = gw * T_B
                        jb = jbases[gi]
                        sadps = sspp.tile([P, GB * T_B, 2], F32, tag="sadps")
                        TPB = 8
                        for k0 in range(0, NJ, TPB):
                            kw = min(TPB, NJ - k0)
                            mps = stpp.tile([P, TPB, P], F16, tag="mps")
                            for k in range(kw):
                                nc.tensor.transpose(out=mps[:, k, :],
                                                    in_=mtall[:, k0 + k, :],
                                                    identity=idn16[:])
                            m2 = sm2p.tile([P, TPB, P], F16, tag="m2sb")
                            if (k0 // TPB) % 2 == 0:
                                nc.vector.tensor_copy(out=m2[:, 0:kw, :],
                                                      in_=mps[:, 0:kw, :])
                            else:
                                nc.scalar.copy(out=m2[:, 0:kw, :],
                                               in_=mps[:, 0:kw, :])
                            for k in range(kw):
                                j = k0 + k
                                blk = g0 + (j // T_LO if j < gw * T_LO
                                            else (j - gw * T_LO) // T_HI)
                                nc.tensor.matmul(
                                    out=sadps[:, j, :], lhsT=m2[:, k, :],
                                    rhs=al2d[:, 2 * blk:2 * blk + 2],
                                    start=True, stop=True)
                        sadsb = sm2p.tile([P, GB * T_B, 2], F32, tag="sadsb")
                        nc.scalar.copy(out=sadsb[:, 0:NJ, :],
                                       in_=sadps[:, 0:NJ, :])
                        nc.vector.tensor_tensor(
                            out=sad_all[:, jb:jb + NJ],
                            in0=AP(sadsb[:].tensor, sadsb[:].offset,
                                   [list(sadsb[:].ap[0]), [2, NJ]]),
                            in1=AP(sadsb[:].tensor, sadsb[:].offset + 1,
                                   [list(sadsb[:].ap[0]), [2, NJ]]),
                            op=ALU.add)

                    mt_live = {0: build_mtall(0)}
                    build_sad(0, mt_live[0])
                    if len(groups) > 1:
                        mt_live[1] = build_mtall(1)
                        build_sad(1, mt_live[1])
                    for gi, (g0, gw) in enumerate(groups):
                        NJ = gw * T_B
                        jbase = jbases[gi]
                        sx = sxp2.tile([P, GB * T_B, R2], F16, tag="sx2")
                        nc.gpsimd.dma_gather(
                            out_ap=sx[:, 0:gw * T_LO, :],
                            in_ap=xp2_tab[0:LO, :],
                            idxs_ap=idxlo[:, g0 * T_LO * 8:(g0 + gw) * T_LO * 8],
                            num_idxs=gw * T_LO * P, num_idxs_reg=gw * T_LO * P,
                            elem_size=R2, single_packet=False)
                        if T_HI:
                            nc.gpsimd.dma_gather(
                                out_ap=sx[:, gw * T_LO:NJ, :],
                                in_ap=xp2_tab[LO:NPAD, :],
                                idxs_ap=idxhi[:, g0 * T_HI * 8:(g0 + gw) * T_HI * 8],
                                num_idxs=gw * T_HI * P, num_idxs_reg=gw * T_HI * P,
                                elem_size=R2, single_packet=False)
                        mtall = mt_live.pop(gi)
                        # scores: (as_hi + as_lo) + sad_all, clamp
                        def col(tile_ap, c):
                            return AP(tile_ap.tensor, tile_ap.offset + c,
                                      [list(tile_ap.ap[0]), [R2, NJ]])
                        zal = smp2.tile([P, GB * T_B], F32, tag="zal")
                        nc.vector.tensor_tensor(out=zal[:, 0:NJ],
                                                in0=col(sx[:], OC),
                                                in1=col(sx[:], OC + 1),
                                                op=ALU.add)
                        ecl2 = smp2.tile([P, GB * T_B], F32, tag="ecl2")
                        nc.vector.tensor_tensor(
                            out=ecl2[:, 0:NJ], in0=zal[:, 0:NJ],
                            in1=sad_all[:, jbase:jbase + NJ], op=ALU.add)
                        nc.vector.tensor_scalar(
                            out=ecl2[:, 0:NJ], in0=ecl2[:, 0:NJ],
                            scalar1=EXP_CLAMP, scalar2=None, op0=ALU.min)
                        lr2 = smp2.tile([P, GB * T_B], F32, tag="lr2")
                        nc.scalar.activation(out=lr2[:, 0:NJ], in_=ecl2[:, 0:NJ],
                                             func=AF.Prelu, alpha=NEG_SLOPE)
                        exd2 = smp2.tile([P, GB * T_B, 2], F16, tag="exd2")
                        for k in range(2):
                            od = AP(exd2[:].tensor, exd2[:].offset + k,
                                    [list(exd2[:].ap[0]), [2, NJ], [1, 1]])
                            nc.scalar.activation(out=od, in_=lr2[:, 0:NJ],
                                                 func=AF.Exp, bias=-1.0)
                        rta = rhp2.tile([P, GB * T_B, OC + 1], F16, tag="rta2")
                        nc.scalar.copy(
                            out=rta[:, 0:NJ, OC],
                            in_=AP(exd2[:].tensor, exd2[:].offset,
                                   [list(exd2[:].ap[0]), [2, NJ]]))
                        in1m = AP(exd2[:].tensor, exd2[:].offset,
                                  [list(exd2[:].ap[0]), [2, NJ],
                                   [0, OC // 2], [1, 2]])
                        nc.vector.tensor_tensor(
                            out=rta[:, 0:NJ, 0:OC], in0=sx[:, 0:NJ, 0:OC],
                            in1=in1m, op=ALU.mult)
                        for b in range(gw):
                            blk = g0 + b
                            psb = psp2.tile([P, OC + 1], F32, tag="psb2")
                            tiles = ([b * T_LO + t for t in range(T_LO)]
                                     + [gw * T_LO + b * T_HI + t
                                        for t in range(T_HI)])
                            for i, j in enumerate(tiles):
                                nc.tensor.matmul(
                                    out=psb[:], lhsT=mtall[:, j, 0:P],
                                    rhs=rta[:, j, 0:OC + 1],
                                    start=(i == 0), stop=(i == len(tiles) - 1))
                            rec = smp2.tile([P, 1], F32, tag="rec2")
                            nc.vector.reciprocal(out=rec[:], in_=psb[:, OC:OC + 1])
                            zb = zp.tile([P, OC], F32, tag="zb")
                            nc.scalar.activation(out=zb[:], in_=psb[:, 0:OC],
                                                 func=AF.Copy, scale=rec[:, 0:1])
                            if cfg.has_b2:
                                nc.vector.tensor_tensor(out=zb[:], in0=zb[:],
                                                        in1=b2t[:], op=ALU.add)
                            nc.sync.dma_start(out=z_t.ap()[blk * P:(blk + 1) * P, :],
                                              in_=zb[:])
                        if gi + 2 < len(groups):
                            mt_live[gi + 2] = build_mtall(gi + 2)
                            build_sad(gi + 2, mt_live[gi + 2])

    nc.compile()
    return nc


_CACHE = {}


def _get_built(cfg):
    key = (cfg, os.environ.get("KNOAG"), os.environ.get("KSHARED"))
    if key not in _CACHE:
        _CACHE[key] = build(cfg)
    return _CACHE[key]


class Runner:
    """Executes the compiled Bass module via PJRT/shard_map with inputs
    pre-sharded per device (no on-device resharding programs)."""

    def __init__(self, nc, n_cores):
        import jax
        from jax.sharding import Mesh, PartitionSpec, NamedSharding
        from jax.experimental.shard_map import shard_map
        from concourse import bass2jax

        bass2jax.install_neuronx_cc_hook()
        self.jax = jax
        self.nc = nc
        self.n_cores = n_cores

        pname = nc.partition_id_tensor.name if nc.partition_id_tensor else None
        in_names, out_names, out_avals = [], [], []
        for alloc in nc.m.functions[0].allocations:
            if not isinstance(alloc, mybir.MemoryLocationSet):
                continue
            name = alloc.memorylocations[0].name
            if alloc.kind == "ExternalInput":
                if name != pname:
                    in_names.append(name)
            elif alloc.kind == "ExternalOutput":
                out_names.append(name)
                out_avals.append(jax.core.ShapedArray(
                    tuple(alloc.tensor_shape), mybir.dt.np(alloc.dtype)))
        self.in_names, self.out_names, self.out_avals = in_names, out_names, out_avals
        all_in = list(in_names) + list(out_names)
        if pname is not None:
            all_in.append(pname)

        def _body(*args):
            operands = list(args)
            if pname is not None:
                operands.append(bass2jax.partition_id_tensor())
            outs = bass2jax._bass_exec_p.bind(
                *operands,
                out_avals=tuple(out_avals),
                in_names=tuple(all_in),
                out_names=tuple(out_names),
                lowering_input_output_aliases=(),
                sim_require_finite=True,
                sim_require_nnan=True,
                nc=nc,
            )
            return tuple(outs)

        self.devices = jax.devices()[:n_cores]
        self.mesh = Mesh(np.asarray(self.devices), ("core",))
        self.sh = NamedSharding(self.mesh, PartitionSpec("core"))
        nspec = (PartitionSpec("core"),)
        self.fn = jax.jit(
            shard_map(_body, mesh=self.mesh,
                      in_specs=nspec * (len(in_names) + len(out_names)),
                      out_specs=nspec * len(out_names), check_rep=False),
            keep_unused=True)
        self.dev_args = None

    def _shard(self, per_core):
        jax = self.jax
        a0 = np.asarray(per_core[0])
        gshape = (self.n_cores * a0.shape[0],) + a0.shape[1:]
        bufs = [jax.device_put(np.asarray(per_core[c]), self.devices[c])
                for c in range(self.n_cores)]
        return jax.make_array_from_single_device_arrays(gshape, self.sh, bufs)

    def set_inputs(self, in_maps):
        args = [self._shard([m[name] for m in in_maps])
                for name in self.in_names]
        for av in self.out_avals:
            z = np.zeros(av.shape, av.dtype)
            args.append(self._shard([z] * self.n_cores))
        self.dev_args = args

    def call(self):
        outs = self.fn(*self.dev_args)
        self.jax.block_until_ready(outs)
        return outs

    def bench(self, k_hi=110, k_lo=10, reps=5):
        """Marginal per-exec time via async-pipelined dispatch: issue k
        back-to-back calls of the single-exec jitted fn, block at the end."""
        import time

        def run_k(k):
            out = None
            for _ in range(k):
                out = self.fn(*self.dev_args)
            self.jax.block_until_ready(out)

        run_k(3)  # warm
        t_lo, t_hi = [], []
        for _ in range(reps):
            t0 = time.perf_counter()
            run_k(k_lo)
            t_lo.append(time.perf_counter() - t0)
            t0 = time.perf_counter()
            run_k(k_hi)
            t_hi.append(time.perf_counter() - t0)
        per_iter = (min(t_hi) - min(t_lo)) / (k_hi - k_lo)
        return per_iter, min(t_lo), min(t_hi)

    def run(self, in_maps):
        self.set_inputs(in_maps)
        outs = self.call()
        res = []
        for c in range(self.n_cores):
            d = {}
            for i, name in enumerate(self.out_names):
                g = np.asarray(outs[i])
                n0 = self.out_avals[i].shape[0]
                d[name] = g.reshape(self.n_cores, n0, *self.out_avals[i].shape[1:])[c]
            res.append(d)
        return res


_RUNNERS = {}


def _get_runner(cfg, nc):
    key = id(nc)
    if key not in _RUNNERS:
        _RUNNERS[key] = Runner(nc, cfg.n_cores)
    return _RUNNERS[key]


def kernel(x, edge_index, W1, a1_src, a1_dst, b1, W2, a2_src, a2_dst, b2):
    x = np.asarray(x)
    cfg, in_maps, pid_of = prep(x, edge_index, W1, a1_src, a1_dst, b1,
                                W2, a2_src, a2_dst, b2)
    nc = _get_built(cfg)
    runner = _get_runner(cfg, nc)
    results = runner.run(in_maps)
    z_full = np.concatenate([results[c]["z"] for c in range(cfg.n_cores)],
                            axis=0)
    return np.ascontiguousarray(z_full[pid_of]).astype(np.float32)


# revision 57
# speedup vs baseline: 1.0490x; 1.0490x over previous
"""Two-layer GAT (PyG GATConv semantics, eval mode) on 8 Trainium2 NeuronCores.

Strategy (dst-sharded, edge-block matmul segment-sum), v2:
  - Host: add self-loops, permute nodes so every 128-node "block" has an
    approximately equal number of incoming edges (snake packing by in-degree),
    assign 49 blocks to each of the 8 cores, group edges by dst block, split
    each block's edges by src < 32768 (int16 gather-index limit), pad each
    group to a fixed tile count. Blocks are processed in groups of GB=2 so
    gathers and element-wise ops batch across blocks.
  - Device, per core (SPMD, one compiled program):
      Phase A: xp = x @ W1 for own nodes (fp16), write to HBM row table.
      AllGather the row table.
      Phase B1 per block-group: one batched dma_gather per src-range (lo/hi),
        one-hot M^T built in ONE tensor_tensor is_equal per group (fp16 iota
        vs dstrel pairs), LeakyReLU+Exp on the Activation engine, messages
        scaled by exp via a pair-duplicated AP (keeps DVE in 2x mode), tensor
        engine accumulates [messages | softmax denom] in PSUM. ELU epilogue
        split across Act (relu/exp parts, scale=1/denom) and one DVE combine.
      Phase C: xp2 = h @ [W2 | W2 a2_src | W2 a2_dst] per own block; row table
        holds [feats fp16 | al2_src hi | al2_src lo]; al2_dst kept per-node in
        SBUF (fp16 hi/lo pair). AllGather.
      Phase B2: same edge machinery, software-pipelined one group ahead;
        per-slot dst scores come from PE transposes of the one-hot (batched
        through PSUM) and tiny matmuls against the al2_dst column instead of
        a per-edge DMA gather (saves ~1/3 of all gather traffic).
  - Host: concat shards, invert the node permutation.
"""

import os
import sys
from dataclasses import dataclass

import numpy as np

for _p in ("/opt/trn_rl_repo", "/root/.axon_site/_ro/trn_rl_repo"):
    if os.path.isdir(_p) and _p not in sys.path:
        sys.path.append(_p)

import concourse.bacc as bacc
import concourse.bass as bass
import concourse.mybir as mybir
import concourse.tile as tile
from concourse import bass_utils
from concourse.ap import AP

F32 = mybir.dt.float32
F16 = mybir.dt.float16  # 2-byte table dtype (fp16: 11-bit mantissa)
I16 = mybir.dt.int16
U16 = mybir.dt.uint16

NEG_SLOPE = 0.2
EXP_CLAMP = 11.4
GB = 2  # blocks per gather/elementwise group


@dataclass(frozen=True)
class GATCfg:
    n_cores: int
    n_pad: int        # padded node count (blocks_total * 128)
    npc: int          # nodes per core
    bpc: int          # blocks per core
    lo_rows: int      # src ids < lo_rows go through the "lo" gather table
    t_lo: int         # tiles of 128 lo-src edges per block
    t_hi: int         # tiles of 128 hi-src edges per block
    in_c: int         # input channels (128)
    hc: int           # heads * hid (256)
    heads: int        # 4
    hid: int          # 64
    out_c: int        # 64
    has_b1: bool
    has_b2: bool

    @property
    def t_b(self):
        return self.t_lo + self.t_hi


def _wrap_idx(arr):
    """dma_gather index layout: linear i -> (partition i%16, col i//16),
    replicated across the 8 Q7 cores (16-partition pattern tiled to 128)."""
    assert arr.size % 16 == 0
    w = arr.reshape(-1, 16).T  # [16, n/16]
    return np.tile(w, (8, 1))  # [128, n/16]


def prep(x, edge_index, W1, a1_src, a1_dst, b1, W2, a2_src, a2_dst, b2,
         n_cores=8, lo_rows_cap=32768):
    N, IN_C = x.shape
    HEADS, HID = a1_src.shape
    HC = HEADS * HID
    OUT_C = W2.shape[1]

    blk_per_core = -(-N // (128 * n_cores))
    npc = blk_per_core * 128
    n_pad = npc * n_cores
    blocks_total = n_pad // 128
    lo_rows = min(lo_rows_cap, n_pad)

    src = np.asarray(edge_index[0], dtype=np.int64)
    dst = np.asarray(edge_index[1], dtype=np.int64)

    # in-degree incl. self-loop, over padded node set
    deg = np.bincount(dst, minlength=n_pad).astype(np.int64) + 1

    # snake-pack nodes into blocks by descending degree -> balanced block loads
    order = np.argsort(-deg, kind="stable")
    rounds = np.arange(n_pad) // blocks_total
    pos = np.arange(n_pad) % blocks_total
    blk_of_sorted = np.where(rounds % 2 == 0, pos, blocks_total - 1 - pos)
    slot_of_sorted = rounds
    pid_of = np.empty(n_pad, dtype=np.int64)
    pid_of[order] = blk_of_sorted * 128 + slot_of_sorted

    # all edges incl self-loops for every (padded) node, in permuted space
    ps = np.concatenate([pid_of[src], np.arange(n_pad)])
    pd = np.concatenate([pid_of[dst], np.arange(n_pad)])
    pd_blk = pd >> 7

    is_lo = ps < lo_rows
    # group edges by (block, hi/lo): sort by block*2 + (1-is_lo)
    gkey = pd_blk * 2 + (~is_lo).astype(np.int64)
    eorder = np.argsort(gkey, kind="stable")
    ps_s, pd_s, key_s = ps[eorder], pd[eorder], gkey[eorder]

    cnt = np.bincount(gkey, minlength=blocks_total * 2)
    cnt_lo = cnt[0::2]
    cnt_hi = cnt[1::2]
    t_lo = int(-(-cnt_lo.max() // 128)) if cnt_lo.max() > 0 else 0
    t_hi = int(-(-cnt_hi.max() // 128)) if cnt_hi.max() > 0 else 0
    if t_hi == 0 and lo_rows < n_pad:
        t_hi = 1
    t_b = t_lo + t_hi
    bpc = blk_per_core

    # per-block slot arrays (block-local tile-major slot order: lo then hi)
    slots = blocks_total * t_b * 128
    slot_ps = np.zeros(slots, dtype=np.int64)          # gather idx (pad 0)
    slot_rel = np.full(slots, -1.0, dtype=np.float32)  # dst_rel (pad -1)
    slot_dst = np.zeros(slots, dtype=np.int64)         # dst id   (pad 0)

    ends = np.cumsum(cnt)
    starts = ends - cnt
    grp = key_s
    within = np.arange(len(ps_s)) - starts[grp]
    base = (grp >> 1) * (t_b * 128) + np.where(grp % 2 == 0, 0, t_lo * 128)
    slot_idx = base + within
    slot_ps[slot_idx] = ps_s
    slot_rel[slot_idx] = (pd_s & 127).astype(np.float32)
    slot_dst[slot_idx] = pd_s

    slot_ps = slot_ps.reshape(n_cores, bpc, t_b * 128)
    slot_rel = slot_rel.reshape(n_cores, bpc, t_b * 128)
    slot_dst = slot_dst.reshape(n_cores, bpc, t_b * 128)

    cfg = GATCfg(n_cores=n_cores, n_pad=n_pad, npc=npc, bpc=bpc,
                 lo_rows=lo_rows, t_lo=t_lo, t_hi=t_hi, in_c=IN_C, hc=HC,
                 heads=HEADS, hid=HID, out_c=OUT_C,
                 has_b1=bool(np.any(np.asarray(b1))),
                 has_b2=bool(np.any(np.asarray(b2))))

    # ---- layer-1 pre-activation scores, exact on host (51 MFLOP) ----
    x32 = np.asarray(x, np.float32)
    W1 = np.asarray(W1, np.float32)
    w1s_h = np.stack([W1[:, h * HID:(h + 1) * HID]
                      @ np.asarray(a1_src, np.float32)[h]
                      for h in range(HEADS)], axis=1)          # [IN_C, H]
    w1d_h = np.stack([W1[:, h * HID:(h + 1) * HID]
                      @ np.asarray(a1_dst, np.float32)[h]
                      for h in range(HEADS)], axis=1)
    als = np.zeros((n_pad, HEADS), np.float32)
    ald = np.zeros((n_pad, HEADS), np.float32)
    als[pid_of[:N]] = x32 @ w1s_h
    ald[pid_of[:N]] = x32 @ w1d_h
    epl_all = np.full((slots, HEADS), -1e4, np.float32)
    epl_all[slot_idx] = np.minimum(als[ps_s] + ald[pd_s], EXP_CLAMP)
    epl_all = epl_all.reshape(n_cores, bpc, t_b * 128, HEADS)

    # ---- node features, transposed + permuted; sharded per core below ----
    xT = np.zeros((IN_C, n_pad), dtype=np.float16)
    xT[:, pid_of[:N]] = np.asarray(x, dtype=np.float16).T

    W2 = np.asarray(W2, np.float32)
    w2s = (W2 @ np.asarray(a2_src, np.float32)[0])[:, None]  # [HC, 1]
    w2d = (W2 @ np.asarray(a2_dst, np.float32)[0])[:, None]
    W2a = np.concatenate([W2, w2s, w2d], axis=1)             # [HC, OUT_C+2]
    c2 = OUT_C + 2
    W2s = np.zeros((128, (HC // 128) * c2), dtype=np.float16)
    for j in range(HC // 128):
        W2s[:, j * c2:(j + 1) * c2] = W2a[j * 128:(j + 1) * 128]

    IOTA16 = np.tile(np.arange(128, dtype=np.float16)[None, :], (128, 1))
    IDN16 = np.eye(128, dtype=np.float16)
    B1 = np.tile(np.asarray(b1, np.float32)[None, :], (128, 1))
    B2 = np.tile(np.asarray(b2, np.float32)[None, :], (128, 1))

    # block-group (GB) reorderings
    n_groups = -(-bpc // GB)
    in_maps = []
    for c in range(n_cores):
        lo_parts, hi_parts = [], []
        epl_parts, drp_parts = [], []
        for g0 in range(0, bpc, GB):
            gw = min(GB, bpc - g0)
            # gather order: all lo tiles of the group's blocks, then all hi
            lo_idx = np.concatenate(
                [slot_ps[c, g0 + b, :t_lo * 128] for b in range(gw)])
            hi_idx = np.concatenate(
                [(slot_ps[c, g0 + b, t_lo * 128:] - lo_rows).clip(min=0)
                 for b in range(gw)])
            lo_parts.append(_wrap_idx(lo_idx.astype(np.int16)))
            if t_hi:
                hi_parts.append(_wrap_idx(hi_idx.astype(np.int16)))
            # group slot order (j_total, lane): lo region then hi region
            epl_g = np.concatenate(
                [epl_all[c, g0 + b, :t_lo * 128] for b in range(gw)]
                + [epl_all[c, g0 + b, t_lo * 128:] for b in range(gw)])
            rel_g = np.concatenate(
                [slot_rel[c, g0 + b, :t_lo * 128] for b in range(gw)]
                + [slot_rel[c, g0 + b, t_lo * 128:] for b in range(gw)])
            # [j, lane] -> [lane, j] transposes: slot linear = j*128 + lane
            n_j = gw * t_b
            epl_parts.append(np.ascontiguousarray(
                epl_g.reshape(n_j, 128, HEADS).transpose(1, 0, 2)
                .reshape(128, n_j * HEADS)))
            rel_l = rel_g.reshape(n_j, 128).T            # [lane, j]
            drp = np.repeat(rel_l, 2, axis=1)            # pairs
            drp_parts.append(drp.astype(np.float16))
        m = {
            "xT": np.ascontiguousarray(xT[:, c * npc:(c + 1) * npc]),
            "W1a": np.asarray(W1, np.float16),
            "W2s": W2s,
            "IOTA16": IOTA16, "IDN16": IDN16,
            "idxlo": np.concatenate(lo_parts, axis=1).astype(np.int16),
            "EPL": np.concatenate(epl_parts, axis=1).astype(np.float32),
            "DRP": np.concatenate(drp_parts, axis=1).astype(np.float16),
        }
        if t_hi:
            m["idxhi"] = np.concatenate(hi_parts, axis=1).astype(np.int16)
        if cfg.has_b1:
            m["B1"] = B1
        if cfg.has_b2:
            m["B2"] = B2
        in_maps.append(m)

    return cfg, in_maps, pid_of[:N]


def build(cfg: GATCfg):
    P = 128
    HC, H, HID, OC = cfg.hc, cfg.heads, cfg.hid, cfg.out_c
    C2 = OC + 2
    T_LO, T_HI, T_B = cfg.t_lo, cfg.t_hi, cfg.t_b
    BPC, NPC, NPAD = cfg.bpc, cfg.npc, cfg.n_pad
    LO = cfg.lo_rows
    R1 = HC            # layer-1 table row width (fp16 elems)
    R2 = 128           # layer-2 table row width (fp16 elems)
    W1COLS = HC + 2 * H  # rhs width in phase B1 (feats + exp + unused pad)

    nc = bacc.Bacc("TRN2", target_bir_lowering=False, debug=False,
                   num_devices=cfg.n_cores)
    xT_t = nc.dram_tensor("xT", [cfg.in_c, NPC], F16, kind="ExternalInput")
    W1a_t = nc.dram_tensor("W1a", [cfg.in_c, HC], F16, kind="ExternalInput")
    W2s_t = nc.dram_tensor("W2s", [P, (HC // P) * C2], F16, kind="ExternalInput")
    IOTA_t = nc.dram_tensor("IOTA16", [P, P], F16, kind="ExternalInput")
    IDN16_t = nc.dram_tensor("IDN16", [P, P], F16, kind="ExternalInput")
    NJ_ALL = sum(min(GB, BPC - g0) * T_B for g0 in range(0, BPC, GB))
    idxlo_t = nc.dram_tensor("idxlo", [P, BPC * T_LO * 8], I16, kind="ExternalInput")
    idxhi_t = (nc.dram_tensor("idxhi", [P, BPC * T_HI * 8], I16, kind="ExternalInput")
               if T_HI else None)
    EPL_t = nc.dram_tensor("EPL", [P, NJ_ALL * H], F32, kind="ExternalInput")
    DRP_t = nc.dram_tensor("DRP", [P, NJ_ALL * 2], F16, kind="ExternalInput")
    B1_t = nc.dram_tensor("B1", [P, HC], F32, kind="ExternalInput") if cfg.has_b1 else None
    B2_t = nc.dram_tensor("B2", [P, OC], F32, kind="ExternalInput") if cfg.has_b2 else None
    z_t = nc.dram_tensor("z", [NPC, OC], F32, kind="ExternalOutput")

    AF = mybir.ActivationFunctionType
    ALU = mybir.AluOpType

    with tile.TileContext(nc) as tc:
        with tc.tile_pool(name="dram", bufs=1, space="DRAM") as dram:
            _shared = "Shared" if os.environ.get("KSHARED", "1") == "1" else "Local"
            xp_own = dram.tile([NPC, R1], F16)
            xp_tab = dram.tile([NPAD, R1], F16, addr_space=_shared)
            xp2_own = dram.tile([NPC, R2], F16)
            xp2_tab = dram.tile([NPAD, R2], F16, addr_space=_shared)

            with tc.tile_pool(name="consts", bufs=1) as consts:
                w1a = consts.tile([P, HC], F16)
                w2s = consts.tile([P, (HC // P) * C2], F16)
                iota = consts.tile([P, P], F16)
                idn16 = consts.tile([P, P], F16)
                shiftc = consts.tile([P, 1], F32)
                nc.vector.memset(shiftc[:], -1.0)
                nc.const_aps.aps[(F32, -1.0)] = shiftc[:]
                nc.sync.dma_start(out=w1a[:], in_=W1a_t.ap())
                nc.sync.dma_start(out=w2s[:], in_=W2s_t.ap())
                nc.sync.dma_start(out=iota[:], in_=IOTA_t.ap())
                nc.sync.dma_start(out=idn16[:], in_=IDN16_t.ap())

                idxlo = consts.tile([P, BPC * T_LO * 8], I16)
                if T_HI:
                    idxhi = consts.tile([P, BPC * T_HI * 8], I16)
                epl = consts.tile([P, NJ_ALL * H], F32)
                drp = consts.tile([P, NJ_ALL * 2], F16)

                def load_b1_consts():
                    nc.sync.dma_start(out=idxlo[:], in_=idxlo_t.ap())
                    if T_HI:
                        nc.sync.dma_start(out=idxhi[:], in_=idxhi_t.ap())
                    nc.sync.dma_start(out=epl[:], in_=EPL_t.ap())
                    nc.sync.dma_start(out=drp[:], in_=DRP_t.ap())
                if cfg.has_b1:
                    b1t = consts.tile([P, HC], F32)
                    nc.sync.dma_start(out=b1t[:], in_=B1_t.ap())
                if cfg.has_b2:
                    b2t = consts.tile([P, OC], F32)
                    nc.sync.dma_start(out=b2t[:], in_=B2_t.ap())

                h_sb = consts.tile([P, BPC * HC], F16)   # layer-1 out (own)
                al2d = consts.tile([P, BPC * 2], F16)    # dst scores (hi,lo)

                # ------- Phase A + B1 (shared SBUF pool context so B1
                # prework overlaps A/AllGather without false WAR deps) -------
                CH = min(8, BPC)
                jbase = 0
                with tc.tile_pool(name="pa_x", bufs=2) as pa_x, \
                     tc.tile_pool(name="pa_o", bufs=2) as pa_o, \
                     tc.tile_pool(name="b1_sx", bufs=2) as sxp, \
                     tc.tile_pool(name="b1_mt", bufs=2) as mtp, \
                     tc.tile_pool(name="b1_rhs", bufs=3) as rhp, \
                     tc.tile_pool(name="b1_sm", bufs=3) as smp, \
                     tc.tile_pool(name="b1_hw", bufs=3) as hwp, \
                     tc.tile_pool(name="c_hT", bufs=4) as chp, \
                     tc.tile_pool(name="c_o", bufs=3) as cop:
                  with tc.tile_pool(name="pa_ps", bufs=4, space="PSUM") as pa_ps:
                    for ch0 in range(0, BPC, CH):
                        cw = min(CH, BPC - ch0)
                        xt = pa_x.tile([P, CH * P], F16, tag="xt")
                        nc.sync.dma_start(
                            out=xt[:, 0:cw * P],
                            in_=xT_t.ap()[:, ch0 * P:(ch0 + cw) * P])
                        ot = pa_o.tile([P, CH * HC], F16, tag="pao")
                        for j in range(cw):
                            ps = pa_ps.tile([P, HC], F32, tag="paps")
                            nc.tensor.matmul(out=ps[:], lhsT=xt[:, j * P:(j + 1) * P],
                                             rhs=w1a[:], start=True, stop=True)
                            if j % 2 == 0:
                                nc.vector.tensor_copy(
                                    out=ot[:, j * HC:(j + 1) * HC], in_=ps[:])
                            else:
                                nc.scalar.copy(
                                    out=ot[:, j * HC:(j + 1) * HC], in_=ps[:])
                        # one batched write: DRAM rows (ch0*P + j*P + p), cols c
                        base = xp_own[ch0 * P:(ch0 + cw) * P, :]
                        odram = AP(base.tensor, base.offset,
                                   [[R1, P], [P * R1, cw], [1, R1]])
                        nc.sync.dma_start(out=odram, in_=ot[:, 0:cw * HC])

                  load_b1_consts()
                  if os.environ.get("KNOAG"):
                    # sim-only stand-in (TimelineSim cannot cost collectives)
                    nc.gpsimd.dma_start(out=xp_tab[0:NPC, :], in_=xp_own[:, :])
                  else:
                    nc.gpsimd.collective_compute(
                        "AllGather", mybir.AluOpType.bypass,
                        ins=[xp_own.opt()],
                        outs=[xp_tab.opt()],
                        replica_groups=[list(range(cfg.n_cores))])

                  # ------- Phase B1 (one-group lookahead: score path +
                  # one-hot for g+1 are emitted before the gather-dependent
                  # mult of g, so the in-order DVE queue never stalls on a
                  # ready-to-run op) -------
                  b1_groups = [(g0, min(GB, BPC - g0)) for g0 in range(0, BPC, GB)]
                  b1_jb = []
                  _jb = 0
                  for g0, gw in b1_groups:
                      b1_jb.append(_jb)
                      _jb += gw * T_B

                  with tc.tile_pool(name="b1_ps", bufs=3, space="PSUM") as psp, \
                       tc.tile_pool(name="b1_hp", bufs=2, space="PSUM") as hpp, \
                       tc.tile_pool(name="c_tp", bufs=1, space="PSUM") as ctp, \
                       tc.tile_pool(name="c_ps", bufs=2, space="PSUM") as cps:
                    def b1_pre(gi):
                        """mtall + score path (independent of gathers)."""
                        g0, gw = b1_groups[gi]
                        NJ = gw * T_B
                        jb = b1_jb[gi]
                        mtall = mtp.tile([P, GB * T_B, P], F16, tag="mt")
                        in0 = AP(iota[:].tensor, iota[:].offset,
                                 [list(iota[:].ap[0]), [0, NJ], [1, P]])
                        in1 = AP(drp[:].tensor, drp[:].offset + jb * 2,
                                 [list(drp[:].ap[0]), [2, NJ], [0, P // 2], [1, 2]])
                        nc.vector.tensor_tensor(out=mtall[:, 0:NJ, :], in0=in0,
                                                in1=in1, op=ALU.is_equal)
                        epl_v = epl[:, jb * H:(jb + NJ) * H].rearrange(
                            "p (j h) -> p j h", j=NJ)
                        lr = smp.tile([P, GB * T_B, H], F32, tag="lr")
                        nc.scalar.activation(out=lr[:, 0:NJ, :], in_=epl_v,
                                             func=AF.Prelu, alpha=NEG_SLOPE)
                        exd = smp.tile([P, GB * T_B, H, 2], F16, tag="exd")
                        for k in range(2):
                            od = AP(exd[:].tensor, exd[:].offset + k,
                                    [list(exd[:].ap[0]), [2 * H, NJ], [2, H],
                                     [1, 1]])
                            nc.scalar.activation(out=od, in_=lr[:, 0:NJ, :],
                                                 func=AF.Exp, bias=-1.0)
                        return mtall, exd

                    pre_live = {0: b1_pre(0)}
                    for gi, (g0, gw) in enumerate(b1_groups):
                        NJ = gw * T_B
                        jbase = b1_jb[gi]
                        sx = sxp.tile([P, GB * T_B, R1], F16, tag="sx")
                        nc.gpsimd.dma_gather(
                            out_ap=sx[:, 0:gw * T_LO, :],
                            in_ap=xp_tab[0:LO, :],
                            idxs_ap=idxlo[:, g0 * T_LO * 8:(g0 + gw) * T_LO * 8],
                            num_idxs=gw * T_LO * P, num_idxs_reg=gw * T_LO * P,
                            elem_size=R1, single_packet=False)
                        if T_HI:
                            nc.gpsimd.dma_gather(
                                out_ap=sx[:, gw * T_LO:NJ, :],
                                in_ap=xp_tab[LO:NPAD, :],
                                idxs_ap=idxhi[:, g0 * T_HI * 8:(g0 + gw) * T_HI * 8],
                                num_idxs=gw * T_HI * P, num_idxs_reg=gw * T_HI * P,
                                elem_size=R1, single_packet=False)
                        if gi + 1 < len(b1_groups):
                            pre_live[gi + 1] = b1_pre(gi + 1)
                        mtall, exd = pre_live.pop(gi)
                        rta = rhp.tile([P, GB * T_B, W1COLS], F16, tag="rta")
                        # exp column for denominator
                        nc.scalar.copy(
                            out=rta[:, 0:NJ, HC:HC + H],
                            in_=AP(exd[:].tensor, exd[:].offset,
                                   [list(exd[:].ap[0]), [2 * H, NJ], [2, H]]))
                        # messages: x_src * exp (pair-duplicated AP keeps
                        # 2x). Emitted per block so the PE can start a block's
                        # matmuls while the next block's messages multiply.
                        blk_ranges = []
                        for b in range(gw):
                            blk_ranges.append((b * T_LO, (b + 1) * T_LO))
                            blk_ranges.append((gw * T_LO + b * T_HI,
                                               gw * T_LO + (b + 1) * T_HI))
                        for (ja, jb_r) in blk_ranges:
                            nw = jb_r - ja
                            in1m = AP(exd[:].tensor, exd[:].offset + ja * 2 * H,
                                      [list(exd[:].ap[0]), [2 * H, nw], [2, H],
                                       [0, HID // 2], [1, 2]])
                            nc.vector.tensor_tensor(
                                out=rta[:, ja:jb_r, 0:HC].rearrange(
                                    "p j (h c) -> p j h c", h=H),
                                in0=sx[:, ja:jb_r, :].rearrange(
                                    "p j (h c) -> p j h c", h=H),
                                in1=in1m, op=ALU.mult)
                        ep = hwp.tile([P, GB * HC], F16, tag="ep")
                        rp = hwp.tile([P, GB * HC], F16, tag="rp")
                        for b in range(gw):
                            blk = g0 + b
                            psb = psp.tile([P, HC + H], F32, tag="psb")
                            tiles = ([b * T_LO + t for t in range(T_LO)]
                                     + [gw * T_LO + b * T_HI + t
                                        for t in range(T_HI)])
                            for i, j in enumerate(tiles):
                                nc.tensor.matmul(
                                    out=psb[:], lhsT=mtall[:, j, 0:P],
                                    rhs=rta[:, j, 0:HC + H],
                                    start=(i == 0), stop=(i == len(tiles) - 1))
                            # epilogue: h = ELU(psum/denom [+ b1])
                            rec = smp.tile([P, H], F32, tag="rec")
                            nc.vector.reciprocal(out=rec[:], in_=psb[:, HC:HC + H])
                            o_rp = rp[:, b * HC:(b + 1) * HC]
                            o_ep = ep[:, b * HC:(b + 1) * HC]
                            if cfg.has_b1:
                                hb = hwp.tile([P, HC], F32, tag="hb")
                                for h in range(H):
                                    nc.scalar.mul(out=hb[:, h * HID:(h + 1) * HID],
                                                  in_=psb[:, h * HID:(h + 1) * HID],
                                                  mul=rec[:, h:h + 1])
                                nc.vector.tensor_tensor(out=hb[:], in0=hb[:],
                                                        in1=b1t[:], op=ALU.add)
                                nc.scalar.activation(out=o_rp, in_=hb[:],
                                                     func=AF.Relu)
                                mn = hwp.tile([P, HC], F32, tag="mn")
                                nc.vector.tensor_scalar(
                                    out=mn[:], in0=hb[:], scalar1=0.0,
                                    scalar2=None, op0=ALU.min)
                                nc.scalar.activation(out=o_ep, in_=mn[:],
                                                     func=AF.Exp)
                            else:
                                # relu(x*rec)=relu(x)*rec; exp(min(x*rec,0)) =
                                # exp(-relu(-x*rec)) -- no DVE min needed
                                nrec = smp.tile([P, H], F32, tag="nrec")
                                nc.vector.tensor_scalar(
                                    out=nrec[:], in0=rec[:], scalar1=-1.0,
                                    scalar2=None, op0=ALU.mult)
                                mn = hwp.tile([P, HC], F16, tag="mn")
                                for h in range(H):
                                    nc.scalar.activation(
                                        out=o_rp[:, h * HID:(h + 1) * HID],
                                        in_=psb[:, h * HID:(h + 1) * HID],
                                        func=AF.Relu, scale=rec[:, h:h + 1])
                                    nc.scalar.activation(
                                        out=mn[:, h * HID:(h + 1) * HID],
                                        in_=psb[:, h * HID:(h + 1) * HID],
                                        func=AF.Relu, scale=nrec[:, h:h + 1])
                                nc.scalar.activation(
                                    out=o_ep, in_=mn[:], func=AF.Exp, scale=-1.0)
                        # h = relu_part + exp_part - 1: accumulate on the
                        # PE (identity stationary), then one Act copy w/ bias
                        hps = hpp.tile([P, GB * HC], F32, tag="hps")
                        nc.tensor.matmul(out=hps[:, 0:gw * HC], lhsT=idn16[:],
                                         rhs=ep[:, 0:gw * HC],
                                         start=True, stop=False)
                        nc.tensor.matmul(out=hps[:, 0:gw * HC], lhsT=idn16[:],
                                         rhs=rp[:, 0:gw * HC],
                                         start=False, stop=True)
                        nc.scalar.activation(
                            out=h_sb[:, g0 * HC:(g0 + gw) * HC],
                            in_=hps[:, 0:gw * HC], func=AF.Copy, bias=-1.0)
                        # ---- fused Phase C for this group's blocks ----
                        o2 = cop.tile([P, GB * R2], F16, tag="o2")
                        for b in range(gw):
                            blk = g0 + b
                            ob = b * R2
                            p2 = cps.tile([P, C2], F32, tag="p2")
                            for j in range(HC // P):
                                pt = ctp.tile([P, P], F16, tag="pt")
                                nc.tensor.transpose(
                                    out=pt[:],
                                    in_=h_sb[:, blk * HC + j * P:
                                             blk * HC + (j + 1) * P],
                                    identity=idn16[:])
                                hT = chp.tile([P, P], F16, tag="hT")
                                nc.scalar.copy(out=hT[:], in_=pt[:])
                                nc.tensor.matmul(
                                    out=p2[:], lhsT=hT[:],
                                    rhs=w2s[:, j * C2:(j + 1) * C2],
                                    start=(j == 0), stop=(j == HC // P - 1))
                            # row: [feats | as_hi | as_lo | 0pad]; the o2
                            # ring buffers keep their pad zeros after the
                            # first cycle, so only zero the first 3 groups
                            if gi < 3:
                                nc.vector.memset(o2[:, ob + OC + 2:ob + R2], 0.0)
                            nc.scalar.copy(out=o2[:, ob:ob + OC + 1],
                                           in_=p2[:, 0:OC + 1])
                            alo = cop.tile([P, 1], F32, tag="alo")
                            nc.vector.tensor_tensor(
                                out=alo[:], in0=p2[:, OC:OC + 1],
                                in1=o2[:, ob + OC:ob + OC + 1],
                                op=ALU.subtract)
                            nc.vector.tensor_copy(
                                out=o2[:, ob + OC + 1:ob + OC + 2], in_=alo[:])
                            nc.scalar.copy(out=al2d[:, 2 * blk:2 * blk + 1],
                                           in_=p2[:, OC + 1:OC + 2])
                            ado = cop.tile([P, 1], F32, tag="ado")
                            nc.vector.tensor_tensor(
                                out=ado[:], in0=p2[:, OC + 1:OC + 2],
                                in1=al2d[:, 2 * blk:2 * blk + 1],
                                op=ALU.subtract)
                            nc.vector.tensor_copy(
                                out=al2d[:, 2 * blk + 1:2 * blk + 2], in_=ado[:])
                        cbase = xp2_own[g0 * P:(g0 + gw) * P, :]
                        codram = AP(cbase.tensor, cbase.offset,
                                    [[R2, P], [P * R2, gw], [1, R2]])
                        nc.sync.dma_start(out=codram, in_=o2[:, 0:gw * R2])

                # ------- sad pre-pass + B2 (shared SBUF pool context) -------
                sad_all = consts.tile([P, NJ_ALL], F32)
                with tc.tile_pool(name="sp_m2", bufs=3) as sm2p, \
                     tc.tile_pool(name="sp_mt", bufs=2) as smtp, \
                     tc.tile_pool(name="b2_sx", bufs=6) as sxp2, \
                     tc.tile_pool(name="b2_mt", bufs=4) as mtp2, \
                     tc.tile_pool(name="b2_rhs", bufs=3) as rhp2, \
                     tc.tile_pool(name="b2_sm", bufs=3) as smp2, \
                     tc.tile_pool(name="b2_z", bufs=3) as zp:
                  if os.environ.get("KNOAG"):
                    nc.gpsimd.dma_start(out=xp2_tab[0:NPC, :], in_=xp2_own[:, :])
                  else:
                    nc.gpsimd.collective_compute(
                        "AllGather", mybir.AluOpType.bypass,
                        ins=[xp2_own.opt()],
                        outs=[xp2_tab.opt()],
                        replica_groups=[list(range(cfg.n_cores))])

                  # ----- B2 with software-pipelined dst-score (sad) -----
                  # Iteration g: issue gathers(g); build mtall(g+1) and its
                  # sad (PE transpose of the one-hot + tiny matmuls vs al2d);
                  # consume mtall(g)/sad_all(g) for scores + aggregation.
                  groups = [(g0, min(GB, BPC - g0)) for g0 in range(0, BPC, GB)]
                  jbases = []
                  _jb = 0
                  for g0, gw in groups:
                      jbases.append(_jb)
                      _jb += gw * T_B

                  with tc.tile_pool(name="sp_tp", bufs=2, space="PSUM") as stpp, \
                       tc.tile_pool(name="sp_sp", bufs=2, space="PSUM") as sspp, \
                       tc.tile_pool(name="b2_ps", bufs=4, space="PSUM") as psp2:

                    def build_mtall(gi):
                        g0, gw = groups[gi]
                        NJ = gw * T_B
                        jb = jbases[gi]
                        mtall = mtp2.tile([P, GB * T_B, P], F16, tag="mt2")
                        in0 = AP(iota[:].tensor, iota[:].offset,
                                 [list(iota[:].ap[0]), [0, NJ], [1, P]])
                        in1 = AP(drp[:].tensor, drp[:].offset + jb * 2,
                                 [list(drp[:].ap[0]), [2, NJ], [0, P // 2], [1, 2]])
                        nc.vector.tensor_tensor(out=mtall[:, 0:NJ, :], in0=in0,
                                                in1=in1, op=ALU.is_equal)
                        return mtall

                    def build_sad(gi, mtall):
                        g0, gw = groups[gi]
                        NJ = gw * T_B
                        jb = jbases[gi]
                        sadps = sspp.tile([P, GB * T_B, 2], F32, tag="sadps")
                        TPB = 8
                        for k0 in range(0, NJ, TPB):
                            kw = min(TPB, NJ - k0)
                            mps = stpp.tile([P, TPB, P], F16, tag="mps")
                            for k in range(kw):
                                nc.tensor.transpose(out=mps[:, k, :],
                                                    in_=mtall[:, k0 + k, :],
                                                    identity=idn16[:])
                            m2 = sm2p.tile([P, TPB, P], F16, tag="m2sb")
                            if (k0 // TPB) % 2 == 0:
                                nc.vector.tensor_copy(out=m2[:, 0:kw, :],
                                                      in_=mps[:, 0:kw, :])
                            else:
                                nc.scalar.copy(out=m2[:, 0:kw, :],
                                               in_=mps[:, 0:kw, :])
                            for k in range(kw):
                                j = k0 + k
                                blk = g0 + (j // T_LO if j < gw * T_LO
                                            else (j - gw * T_LO) // T_HI)
                                nc.tensor.matmul(
                                    out=sadps[:, j, :], lhsT=m2[:, k, :],
                                    rhs=al2d[:, 2 * blk:2 * blk + 2],
                                    start=True, stop=True)
                        sadsb = sm2p.tile([P, GB * T_B, 2], F32, tag="sadsb")
                        nc.scalar.copy(out=sadsb[:, 0:NJ, :],
                                       in_=sadps[:, 0:NJ, :])
                        nc.vector.tensor_tensor(
                            out=sad_all[:, jb:jb + NJ],
                            in0=AP(sadsb[:].tensor, sadsb[:].offset,
                                   [list(sadsb[:].ap[0]), [2, NJ]]),
                            in1=AP(sadsb[:].tensor, sadsb[:].offset + 1,
                                   [list(sadsb[:].ap[0]), [2, NJ]]),
                            op=ALU.add)

                    mt_live = {0: build_mtall(0)}
                    build_sad(0, mt_live[0])
                    if len(groups) > 1:
                        mt_live[1] = build_mtall(1)
                        build_sad(1, mt_live[1])
                    for gi, (g0, gw) in enumerate(groups):
                        NJ = gw * T_B
                        jbase = jbases[gi]
                        sx = sxp2.tile([P, GB * T_B, R2], F16, tag="sx2")
                        nc.gpsimd.dma_gather(
                            out_ap=sx[:, 0:gw * T_LO, :],
                            in_ap=xp2_tab[0:LO, :],
                            idxs_ap=idxlo[:, g0 * T_LO * 8:(g0 + gw) * T_LO * 8],
                            num_idxs=gw * T_LO * P, num_idxs_reg=gw * T_LO * P,
                            elem_size=R2, single_packet=False)
                        if T_HI:
                            nc.gpsimd.dma_gather(
                                out_ap=sx[:, gw * T_LO:NJ, :],
                                in_ap=xp2_tab[LO:NPAD, :],
                                idxs_ap=idxhi[:, g0 * T_HI * 8:(g0 + gw) * T_HI * 8],
                                num_idxs=gw * T_HI * P, num_idxs_reg=gw * T_HI * P,
                                elem_size=R2, single_packet=False)
                        mtall = mt_live.pop(gi)
                        # scores: (as_hi + as_lo) + sad_all, clamp
                        def col(tile_ap, c):
                            return AP(tile_ap.tensor, tile_ap.offset + c,
                                      [list(tile_ap.ap[0]), [R2, NJ]])
                        zal = smp2.tile([P, GB * T_B], F32, tag="zal")
                        nc.vector.tensor_tensor(out=zal[:, 0:NJ],
                                                in0=col(sx[:], OC),
                                                in1=col(sx[:], OC + 1),
                                                op=ALU.add)
                        ecl2 = smp2.tile([P, GB * T_B], F32, tag="ecl2")
                        nc.vector.tensor_tensor(
                            out=ecl2[:, 0:NJ], in0=zal[:, 0:NJ],
                            in1=sad_all[:, jbase:jbase + NJ], op=ALU.add)
                        nc.vector.tensor_scalar(
                            out=ecl2[:, 0:NJ], in0=ecl2[:, 0:NJ],
                            scalar1=EXP_CLAMP, scalar2=None, op0=ALU.min)
                        lr2 = smp2.tile([P, GB * T_B], F32, tag="lr2")
                        nc.scalar.activation(out=lr2[:, 0:NJ], in_=ecl2[:, 0:NJ],
                                             func=AF.Prelu, alpha=NEG_SLOPE)
                        exd2 = smp2.tile([P, GB * T_B, 2], F16, tag="exd2")
                        for k in range(2):
                            od = AP(exd2[:].tensor, exd2[:].offset + k,
                                    [list(exd2[:].ap[0]), [2, NJ], [1, 1]])
                            nc.scalar.activation(out=od, in_=lr2[:, 0:NJ],
                                                 func=AF.Exp, bias=-1.0)
                        rta = rhp2.tile([P, GB * T_B, OC + 1], F16, tag="rta2")
                        nc.scalar.copy(
                            out=rta[:, 0:NJ, OC],
                            in_=AP(exd2[:].tensor, exd2[:].offset,
                                   [list(exd2[:].ap[0]), [2, NJ]]))
                        in1m = AP(exd2[:].tensor, exd2[:].offset,
                                  [list(exd2[:].ap[0]), [2, NJ],
                                   [0, OC // 2], [1, 2]])
                        nc.vector.tensor_tensor(
                            out=rta[:, 0:NJ, 0:OC], in0=sx[:, 0:NJ, 0:OC],
                            in1=in1m, op=ALU.mult)
                        for b in range(gw):
                            blk = g0 + b
                            psb = psp2.tile([P, OC + 1], F32, tag="psb2")
                            tiles = ([b * T_LO + t for t in range(T_LO)]
                                     + [gw * T_LO + b * T_HI + t
                                        for t in range(T_HI)])
                            for i, j in enumerate(tiles):
                                nc.tensor.matmul(
                                    out=psb[:], lhsT=mtall[:, j, 0:P],
                                    rhs=rta[:, j, 0:OC + 1],
                                    start=(i == 0), stop=(i == len(tiles) - 1))
                            rec = smp2.tile([P, 1], F32, tag="rec2")
                            nc.vector.reciprocal(out=rec[:], in_=psb[:, OC:OC + 1])
                            zb = zp.tile([P, OC], F32, tag="zb")
                            nc.scalar.activation(out=zb[:], in_=psb[:, 0:OC],
                                                 func=AF.Copy, scale=rec[:, 0:1])
                            if cfg.has_b2:
                                nc.vector.tensor_tensor(out=zb[:], in0=zb[:],
                                                        in1=b2t[:], op=ALU.add)
                            nc.sync.dma_start(out=z_t.ap()[blk * P:(blk + 1) * P, :],
                                              in_=zb[:])
                        if gi + 2 < len(groups):
                            mt_live[gi + 2] = build_mtall(gi + 2)
                            build_sad(gi + 2, mt_live[gi + 2])

    nc.compile()
    return nc


_CACHE = {}


def _get_built(cfg):
    key = (cfg, os.environ.get("KNOAG"), os.environ.get("KSHARED"))
    if key not in _CACHE:
        _CACHE[key] = build(cfg)
    return _CACHE[key]


class Runner:
    """Executes the compiled Bass module via PJRT/shard_map with inputs
    pre-sharded per device (no on-device resharding programs)."""

    def __init__(self, nc, n_cores):
        import jax
        from jax.sharding import Mesh, PartitionSpec, NamedSharding
        from jax.experimental.shard_map import shard_map
        from concourse import bass2jax

        bass2jax.install_neuronx_cc_hook()
        self.jax = jax
        self.nc = nc
        self.n_cores = n_cores

        pname = nc.partition_id_tensor.name if nc.partition_id_tensor else None
        in_names, out_names, out_avals = [], [], []
        for alloc in nc.m.functions[0].allocations:
            if not isinstance(alloc, mybir.MemoryLocationSet):
                continue
            name = alloc.memorylocations[0].name
            if alloc.kind == "ExternalInput":
                if name != pname:
                    in_names.append(name)
            elif alloc.kind == "ExternalOutput":
                out_names.append(name)
                out_avals.append(jax.core.ShapedArray(
                    tuple(alloc.tensor_shape), mybir.dt.np(alloc.dtype)))
        self.in_names, self.out_names, self.out_avals = in_names, out_names, out_avals
        all_in = list(in_names) + list(out_names)
        if pname is not None:
            all_in.append(pname)

        def _body(*args):
            operands = list(args)
            if pname is not None:
                operands.append(bass2jax.partition_id_tensor())
            outs = bass2jax._bass_exec_p.bind(
                *operands,
                out_avals=tuple(out_avals),
                in_names=tuple(all_in),
                out_names=tuple(out_names),
                lowering_input_output_aliases=(),
                sim_require_finite=True,
                sim_require_nnan=True,
                nc=nc,
            )
            return tuple(outs)

        self.devices = jax.devices()[:n_cores]
        self.mesh = Mesh(np.asarray(self.devices), ("core",))
        self.sh = NamedSharding(self.mesh, PartitionSpec("core"))
        nspec = (PartitionSpec("core"),)
        self.fn = jax.jit(
            shard_map(_body, mesh=self.mesh,
                      in_specs=nspec * (len(in_names) + len(out_names)),
                      out_specs=nspec * len(out_names), check_rep=False),
            keep_unused=True)
        self.dev_args = None

    def _shard(self, per_core):
        jax = self.jax
        a0 = np.asarray(per_core[0])
        gshape = (self.n_cores * a0.shape[0],) + a0.shape[1:]
        bufs = [jax.device_put(np.asarray(per_core[c]), self.devices[c])
                for c in range(self.n_cores)]
        return jax.make_array_from_single_device_arrays(gshape, self.sh, bufs)

    def set_inputs(self, in_maps):
        args = [self._shard([m[name] for m in in_maps])
                for name in self.in_names]
        for av in self.out_avals:
            z = np.zeros(av.shape, av.dtype)
            args.append(self._shard([z] * self.n_cores))
        self.dev_args = args

    def call(self):
        outs = self.fn(*self.dev_args)
        self.jax.block_until_ready(outs)
        return outs

    def bench(self, k_hi=110, k_lo=10, reps=5):
        """Marginal per-exec time via async-pipelined dispatch: issue k
        back-to-back calls of the single-exec jitted fn, block at the end."""
        import time

        def run_k(k):
            out = None
            for _ in range(k):
                out = self.fn(*self.dev_args)
            self.jax.block_until_ready(out)

        run_k(3)  # warm
        t_lo, t_hi = [], []
        for _ in range(reps):
            t0 = time.perf_counter()
            run_k(k_lo)
            t_lo.append(time.perf_counter() - t0)
            t0 = time.perf_counter()
            run_k(k_hi)
            t_hi.append(time.perf_counter() - t0)
        per_iter = (min(t_hi) - min(t_lo)) / (k_hi - k_lo)
        return per_iter, min(t_lo), min(t_hi)

    def run(self, in_maps):
        self.set_inputs(in_maps)
        outs = self.call()
        res = []
        for c in range(self.n_cores):
            d = {}
            for i, name in enumerate(self.out_names):
                g = np.asarray(outs[i])
                n0 = self.out_avals[i].shape[0]
                d[name] = g.reshape(self.n_cores, n0, *self.out_avals[i].shape[1:])[c]
            res.append(d)
        return res


_RUNNERS = {}


def _get_runner(cfg, nc):
    key = id(nc)
    if key not in _RUNNERS:
        _RUNNERS[key] = Runner(nc, cfg.n_cores)
    return _RUNNERS[key]


def kernel(x, edge_index, W1, a1_src, a1_dst, b1, W2, a2_src, a2_dst, b2):
    x = np.asarray(x)
    cfg, in_maps, pid_of = prep(x, edge_index, W1, a1_src, a1_dst, b1,
                                W2, a2_src, a2_dst, b2)
    nc = _get_built(cfg)
    runner = _get_runner(cfg, nc)
    results = runner.run(in_maps)
    z_full = np.concatenate([results[c]["z"] for c in range(cfg.n_cores)],
                            axis=0)
    return np.ascontiguousarray(z_full[pid_of]).astype(np.float32)


# revision 61
# speedup vs baseline: 1.4555x; 1.3876x over previous
"""Two-layer GAT (PyG GATConv semantics, eval mode) on 8 Trainium2 NeuronCores.

Strategy (dst-sharded, edge-block matmul segment-sum), v2:
  - Host: add self-loops, permute nodes so every 128-node "block" has an
    approximately equal number of incoming edges (snake packing by in-degree),
    assign 49 blocks to each of the 8 cores, group edges by dst block, split
    each block's edges by src < 32768 (int16 gather-index limit), pad each
    group to a fixed tile count. Blocks are processed in groups of GB=2 so
    gathers and element-wise ops batch across blocks.
  - Device, per core (SPMD, one compiled program):
      Phase A: xp = x @ W1 for own nodes (fp16), write to HBM row table.
      AllGather the row table.
      Phase B1 per block-group: one batched dma_gather per src-range (lo/hi),
        one-hot M^T built in ONE tensor_tensor is_equal per group (fp16 iota
        vs dstrel pairs), LeakyReLU+Exp on the Activation engine, messages
        scaled by exp via a pair-duplicated AP (keeps DVE in 2x mode), tensor
        engine accumulates [messages | softmax denom] in PSUM. ELU epilogue
        split across Act (relu/exp parts, scale=1/denom) and one DVE combine.
      Phase C: xp2 = h @ [W2 | W2 a2_src | W2 a2_dst] per own block; row table
        holds [feats fp16 | al2_src hi | al2_src lo]; al2_dst kept per-node in
        SBUF (fp16 hi/lo pair). AllGather.
      Phase B2: same edge machinery, software-pipelined one group ahead;
        per-slot dst scores come from PE transposes of the one-hot (batched
        through PSUM) and tiny matmuls against the al2_dst column instead of
        a per-edge DMA gather (saves ~1/3 of all gather traffic).
  - Host: concat shards, invert the node permutation.
"""

import os
import sys
from dataclasses import dataclass

import numpy as np

for _p in ("/opt/trn_rl_repo", "/root/.axon_site/_ro/trn_rl_repo"):
    if os.path.isdir(_p) and _p not in sys.path:
        sys.path.append(_p)

import concourse.bacc as bacc
import concourse.bass as bass
import concourse.mybir as mybir
import concourse.tile as tile
from concourse import bass_utils
from concourse.ap import AP

F32 = mybir.dt.float32
F16 = mybir.dt.float16  # 2-byte table dtype (fp16: 11-bit mantissa)
I16 = mybir.dt.int16
U16 = mybir.dt.uint16

NEG_SLOPE = 0.2
EXP_CLAMP = 11.4
GB = 2  # blocks per gather/elementwise group


@dataclass(frozen=True)
class GATCfg:
    n_cores: int
    n_pad: int        # padded node count (blocks_total * 128)
    npc: int          # nodes per core
    bpc: int          # blocks per core
    lo_rows: int      # src ids < lo_rows go through the "lo" gather table
    t_lo: int         # tiles of 128 lo-src edges per block
    t_hi: int         # tiles of 128 hi-src edges per block
    in_c: int         # input channels (128)
    hc: int           # heads * hid (256)
    heads: int        # 4
    hid: int          # 64
    out_c: int        # 64
    has_b1: bool
    has_b2: bool

    @property
    def t_b(self):
        return self.t_lo + self.t_hi


def _wrap_idx(arr):
    """dma_gather index layout: linear i -> (partition i%16, col i//16),
    replicated across the 8 Q7 cores (16-partition pattern tiled to 128)."""
    assert arr.size % 16 == 0
    w = arr.reshape(-1, 16).T  # [16, n/16]
    return np.tile(w, (8, 1))  # [128, n/16]


def prep(x, edge_index, W1, a1_src, a1_dst, b1, W2, a2_src, a2_dst, b2,
         n_cores=8, lo_rows_cap=32768):
    N, IN_C = x.shape
    HEADS, HID = a1_src.shape
    HC = HEADS * HID
    OUT_C = W2.shape[1]

    blk_per_core = -(-N // (128 * n_cores))
    npc = blk_per_core * 128
    n_pad = npc * n_cores
    blocks_total = n_pad // 128
    lo_rows = min(lo_rows_cap, n_pad)

    src = np.asarray(edge_index[0], dtype=np.int64)
    dst = np.asarray(edge_index[1], dtype=np.int64)

    # in-degree (self-loops handled separately as a per-lane scalar term)
    deg = np.bincount(dst, minlength=n_pad).astype(np.int64)

    # snake-pack nodes into blocks by descending degree -> balanced block loads
    order = np.argsort(-deg, kind="stable")
    rounds = np.arange(n_pad) // blocks_total
    pos = np.arange(n_pad) % blocks_total
    blk_of_sorted = np.where(rounds % 2 == 0, pos, blocks_total - 1 - pos)
    slot_of_sorted = rounds
    pid_of = np.empty(n_pad, dtype=np.int64)
    pid_of[order] = blk_of_sorted * 128 + slot_of_sorted

    # real edges only, in permuted space (self-loops applied separately)
    ps = pid_of[src]
    pd = pid_of[dst]
    pd_blk = pd >> 7

    is_lo = ps < lo_rows
    # group edges by (block, hi/lo): sort by block*2 + (1-is_lo)
    gkey = pd_blk * 2 + (~is_lo).astype(np.int64)
    eorder = np.argsort(gkey, kind="stable")
    ps_s, pd_s, key_s = ps[eorder], pd[eorder], gkey[eorder]

    cnt = np.bincount(gkey, minlength=blocks_total * 2)
    cnt_lo = cnt[0::2]
    cnt_hi = cnt[1::2]
    t_lo = int(-(-cnt_lo.max() // 128)) if cnt_lo.max() > 0 else 0
    t_hi = int(-(-cnt_hi.max() // 128)) if cnt_hi.max() > 0 else 0
    if t_hi == 0 and lo_rows < n_pad:
        t_hi = 1
    t_b = t_lo + t_hi
    bpc = blk_per_core

    # per-block slot arrays (block-local tile-major slot order: lo then hi)
    slots = blocks_total * t_b * 128
    slot_ps = np.zeros(slots, dtype=np.int64)          # gather idx (pad 0)
    slot_rel = np.full(slots, -1.0, dtype=np.float32)  # dst_rel (pad -1)
    slot_dst = np.zeros(slots, dtype=np.int64)         # dst id   (pad 0)

    ends = np.cumsum(cnt)
    starts = ends - cnt
    grp = key_s
    within = np.arange(len(ps_s)) - starts[grp]
    base = (grp >> 1) * (t_b * 128) + np.where(grp % 2 == 0, 0, t_lo * 128)
    slot_idx = base + within
    slot_ps[slot_idx] = ps_s
    slot_rel[slot_idx] = (pd_s & 127).astype(np.float32)
    slot_dst[slot_idx] = pd_s

    slot_ps = slot_ps.reshape(n_cores, bpc, t_b * 128)
    slot_rel = slot_rel.reshape(n_cores, bpc, t_b * 128)
    slot_dst = slot_dst.reshape(n_cores, bpc, t_b * 128)

    cfg = GATCfg(n_cores=n_cores, n_pad=n_pad, npc=npc, bpc=bpc,
                 lo_rows=lo_rows, t_lo=t_lo, t_hi=t_hi, in_c=IN_C, hc=HC,
                 heads=HEADS, hid=HID, out_c=OUT_C,
                 has_b1=bool(np.any(np.asarray(b1))),
                 has_b2=bool(np.any(np.asarray(b2))))

    # ---- layer-1 pre-activation scores, exact on host (51 MFLOP) ----
    x32 = np.asarray(x, np.float32)
    W1 = np.asarray(W1, np.float32)
    w1s_h = np.stack([W1[:, h * HID:(h + 1) * HID]
                      @ np.asarray(a1_src, np.float32)[h]
                      for h in range(HEADS)], axis=1)          # [IN_C, H]
    w1d_h = np.stack([W1[:, h * HID:(h + 1) * HID]
                      @ np.asarray(a1_dst, np.float32)[h]
                      for h in range(HEADS)], axis=1)
    als = np.zeros((n_pad, HEADS), np.float32)
    ald = np.zeros((n_pad, HEADS), np.float32)
    als[pid_of[:N]] = x32 @ w1s_h
    ald[pid_of[:N]] = x32 @ w1d_h
    epl_all = np.full((slots, HEADS), -1e4, np.float32)
    epl_all[slot_idx] = np.minimum(als[ps_s] + ald[pd_s], EXP_CLAMP)
    epl_all = epl_all.reshape(n_cores, bpc, t_b * 128, HEADS)
    # self-loop factors, exact on host: exp(lrelu(als+ald)) per node
    e_self = np.minimum(als + ald, EXP_CLAMP)
    e_self = np.where(e_self >= 0, e_self, NEG_SLOPE * e_self)
    exp_self = np.exp(e_self - 1.0).astype(np.float32)  # matches Exp bias=-1
    # [n_pad, H] -> per core [128 lanes, bpc*H]
    es = exp_self.reshape(n_cores, bpc, 128, HEADS).transpose(0, 2, 1, 3)

    # ---- node features, transposed + permuted; sharded per core below ----
    xT = np.zeros((IN_C, n_pad), dtype=np.float16)
    xT[:, pid_of[:N]] = np.asarray(x, dtype=np.float16).T

    W2 = np.asarray(W2, np.float32)
    w2s = (W2 @ np.asarray(a2_src, np.float32)[0])[:, None]  # [HC, 1]
    w2d = (W2 @ np.asarray(a2_dst, np.float32)[0])[:, None]
    W2a = np.concatenate([W2, w2s, w2d], axis=1)             # [HC, OUT_C+2]
    c2 = OUT_C + 2
    W2s = np.zeros((128, (HC // 128) * c2), dtype=np.float16)
    for j in range(HC // 128):
        W2s[:, j * c2:(j + 1) * c2] = W2a[j * 128:(j + 1) * 128]

    IOTA16 = np.tile(np.arange(128, dtype=np.float16)[None, :], (128, 1))
    IDN16 = np.eye(128, dtype=np.float16)
    B1 = np.tile(np.asarray(b1, np.float32)[None, :], (128, 1))
    B2 = np.tile(np.asarray(b2, np.float32)[None, :], (128, 1))

    # block-group (GB) reorderings
    n_groups = -(-bpc // GB)
    in_maps = []
    for c in range(n_cores):
        lo_parts, hi_parts = [], []
        epl_parts, drp_parts = [], []
        for g0 in range(0, bpc, GB):
            gw = min(GB, bpc - g0)
            # gather order: all lo tiles of the group's blocks, then all hi
            lo_idx = np.concatenate(
                [slot_ps[c, g0 + b, :t_lo * 128] for b in range(gw)])
            hi_idx = np.concatenate(
                [(slot_ps[c, g0 + b, t_lo * 128:] - lo_rows).clip(min=0)
                 for b in range(gw)])
            lo_parts.append(_wrap_idx(lo_idx.astype(np.int16)))
            if t_hi:
                hi_parts.append(_wrap_idx(hi_idx.astype(np.int16)))
            # group slot order (j_total, lane): lo region then hi region
            epl_g = np.concatenate(
                [epl_all[c, g0 + b, :t_lo * 128] for b in range(gw)]
                + [epl_all[c, g0 + b, t_lo * 128:] for b in range(gw)])
            rel_g = np.concatenate(
                [slot_rel[c, g0 + b, :t_lo * 128] for b in range(gw)]
                + [slot_rel[c, g0 + b, t_lo * 128:] for b in range(gw)])
            # [j, lane] -> [lane, j] transposes: slot linear = j*128 + lane
            n_j = gw * t_b
            epl_parts.append(np.ascontiguousarray(
                epl_g.reshape(n_j, 128, HEADS).transpose(1, 0, 2)
                .reshape(128, n_j * HEADS)))
            rel_l = rel_g.reshape(n_j, 128).T            # [lane, j]
            drp = np.repeat(rel_l, 2, axis=1)            # pairs
            drp_parts.append(drp.astype(np.float16))
        m = {
            "ESELF": np.ascontiguousarray(
                es[c].reshape(128, bpc * HEADS)).astype(np.float32),
            "xT": np.ascontiguousarray(xT[:, c * npc:(c + 1) * npc]),
            "W1a": np.asarray(W1, np.float16),
            "W2s": W2s,
            "IOTA16": IOTA16, "IDN16": IDN16,
            "idxlo": np.concatenate(lo_parts, axis=1).astype(np.int16),
            "EPL": np.concatenate(epl_parts, axis=1).astype(np.float32),
            "DRP": np.concatenate(drp_parts, axis=1).astype(np.float16),
        }
        if t_hi:
            m["idxhi"] = np.concatenate(hi_parts, axis=1).astype(np.int16)
        if cfg.has_b1:
            m["B1"] = B1
        if cfg.has_b2:
            m["B2"] = B2
        in_maps.append(m)

    return cfg, in_maps, pid_of[:N]


def build(cfg: GATCfg):
    P = 128
    HC, H, HID, OC = cfg.hc, cfg.heads, cfg.hid, cfg.out_c
    C2 = OC + 2
    T_LO, T_HI, T_B = cfg.t_lo, cfg.t_hi, cfg.t_b
    BPC, NPC, NPAD = cfg.bpc, cfg.npc, cfg.n_pad
    LO = cfg.lo_rows
    R1 = HC            # layer-1 table row width (fp16 elems)
    R2 = 128           # layer-2 table row width (fp16 elems)
    W1COLS = HC + 2 * H  # rhs width in phase B1 (feats + exp + unused pad)

    nc = bacc.Bacc("TRN2", target_bir_lowering=False, debug=False,
                   num_devices=cfg.n_cores)
    xT_t = nc.dram_tensor("xT", [cfg.in_c, NPC], F16, kind="ExternalInput")
    W1a_t = nc.dram_tensor("W1a", [cfg.in_c, HC], F16, kind="ExternalInput")
    W2s_t = nc.dram_tensor("W2s", [P, (HC // P) * C2], F16, kind="ExternalInput")
    IOTA_t = nc.dram_tensor("IOTA16", [P, P], F16, kind="ExternalInput")
    IDN16_t = nc.dram_tensor("IDN16", [P, P], F16, kind="ExternalInput")
    NJ_ALL = sum(min(GB, BPC - g0) * T_B for g0 in range(0, BPC, GB))
    idxlo_t = nc.dram_tensor("idxlo", [P, BPC * T_LO * 8], I16, kind="ExternalInput")
    idxhi_t = (nc.dram_tensor("idxhi", [P, BPC * T_HI * 8], I16, kind="ExternalInput")
               if T_HI else None)
    EPL_t = nc.dram_tensor("EPL", [P, NJ_ALL * H], F32, kind="ExternalInput")
    ESELF_t = nc.dram_tensor("ESELF", [P, BPC * H], F32, kind="ExternalInput")
    DRP_t = nc.dram_tensor("DRP", [P, NJ_ALL * 2], F16, kind="ExternalInput")
    B1_t = nc.dram_tensor("B1", [P, HC], F32, kind="ExternalInput") if cfg.has_b1 else None
    B2_t = nc.dram_tensor("B2", [P, OC], F32, kind="ExternalInput") if cfg.has_b2 else None
    z_t = nc.dram_tensor("z", [NPC, OC], F32, kind="ExternalOutput")

    AF = mybir.ActivationFunctionType
    ALU = mybir.AluOpType

    with tile.TileContext(nc) as tc:
        with tc.tile_pool(name="dram", bufs=1, space="DRAM") as dram:
            _shared = "Shared" if os.environ.get("KSHARED", "1") == "1" else "Local"
            xp_own = dram.tile([NPC, R1], F16)
            xp_tab = dram.tile([NPAD, R1], F16, addr_space=_shared)
            xp2_own = dram.tile([NPC, R2], F16)
            xp2_tab = dram.tile([NPAD, R2], F16, addr_space=_shared)

            with tc.tile_pool(name="consts", bufs=1) as consts:
                w1a = consts.tile([P, HC], F16)
                w2s = consts.tile([P, (HC // P) * C2], F16)
                iota = consts.tile([P, P], F16)
                idn16 = consts.tile([P, P], F16)
                shiftc = consts.tile([P, 1], F32)
                nc.vector.memset(shiftc[:], -1.0)
                nc.const_aps.aps[(F32, -1.0)] = shiftc[:]
                nc.sync.dma_start(out=w1a[:], in_=W1a_t.ap())
                nc.sync.dma_start(out=w2s[:], in_=W2s_t.ap())
                nc.sync.dma_start(out=iota[:], in_=IOTA_t.ap())
                nc.sync.dma_start(out=idn16[:], in_=IDN16_t.ap())

                idxlo = consts.tile([P, BPC * T_LO * 8], I16)
                if T_HI:
                    idxhi = consts.tile([P, BPC * T_HI * 8], I16)
                epl = consts.tile([P, NJ_ALL * H], F32)
                eself = consts.tile([P, BPC * H], F32)
                drp = consts.tile([P, NJ_ALL * 2], F16)

                def load_b1_consts():
                    nc.sync.dma_start(out=idxlo[:], in_=idxlo_t.ap())
                    if T_HI:
                        nc.sync.dma_start(out=idxhi[:], in_=idxhi_t.ap())
                    nc.sync.dma_start(out=epl[:], in_=EPL_t.ap())
                    nc.sync.dma_start(out=eself[:], in_=ESELF_t.ap())
                    nc.sync.dma_start(out=drp[:], in_=DRP_t.ap())
                if cfg.has_b1:
                    b1t = consts.tile([P, HC], F32)
                    nc.sync.dma_start(out=b1t[:], in_=B1_t.ap())
                if cfg.has_b2:
                    b2t = consts.tile([P, OC], F32)
                    nc.sync.dma_start(out=b2t[:], in_=B2_t.ap())

                h_sb = consts.tile([P, BPC * HC], F16)   # layer-1 out (own)
                al2d = consts.tile([P, BPC * 2], F16)    # dst scores (hi,lo)
                al2s = consts.tile([P, BPC * 2], F16)    # src scores (hi,lo)

                # ------- Phase A + B1 (shared SBUF pool context so B1
                # prework overlaps A/AllGather without false WAR deps) -------
                CH = min(8, BPC)
                jbase = 0
                with tc.tile_pool(name="pa_x", bufs=1) as pa_x, \
                     tc.tile_pool(name="pa_o", bufs=2) as pa_o, \
                     tc.tile_pool(name="b1_sx", bufs=2) as sxp, \
                     tc.tile_pool(name="b1_mt", bufs=2) as mtp, \
                     tc.tile_pool(name="b1_rhs", bufs=3) as rhp, \
                     tc.tile_pool(name="b1_sm", bufs=3) as smp, \
                     tc.tile_pool(name="b1_hw", bufs=3) as hwp, \
                     tc.tile_pool(name="c_hT", bufs=4) as chp, \
                     tc.tile_pool(name="c_o", bufs=3) as cop, \
                     tc.tile_pool(name="b1_sf", bufs=2) as sfp:
                  with tc.tile_pool(name="pa_ps", bufs=4, space="PSUM") as pa_ps:
                    for ch0 in range(0, BPC, CH):
                        cw = min(CH, BPC - ch0)
                        xt = pa_x.tile([P, CH * P], F16, tag="xt")
                        nc.sync.dma_start(
                            out=xt[:, 0:cw * P],
                            in_=xT_t.ap()[:, ch0 * P:(ch0 + cw) * P])
                        ot = pa_o.tile([P, CH * HC], F16, tag="pao")
                        for j in range(cw):
                            ps = pa_ps.tile([P, HC], F32, tag="paps")
                            nc.tensor.matmul(out=ps[:], lhsT=xt[:, j * P:(j + 1) * P],
                                             rhs=w1a[:], start=True, stop=True)
                            if j % 2 == 0:
                                nc.vector.tensor_copy(
                                    out=ot[:, j * HC:(j + 1) * HC], in_=ps[:])
                            else:
                                nc.scalar.copy(
                                    out=ot[:, j * HC:(j + 1) * HC], in_=ps[:])
                        # one batched write: DRAM rows (ch0*P + j*P + p), cols c
                        base = xp_own[ch0 * P:(ch0 + cw) * P, :]
                        odram = AP(base.tensor, base.offset,
                                   [[R1, P], [P * R1, cw], [1, R1]])
                        nc.sync.dma_start(out=odram, in_=ot[:, 0:cw * HC])

                  load_b1_consts()
                  if os.environ.get("KNOAG"):
                    # sim-only stand-in (TimelineSim cannot cost collectives)
                    nc.gpsimd.dma_start(out=xp_tab[0:NPC, :], in_=xp_own[:, :])
                  else:
                    nc.gpsimd.collective_compute(
                        "AllGather", mybir.AluOpType.bypass,
                        ins=[xp_own.opt()],
                        outs=[xp_tab.opt()],
                        replica_groups=[list(range(cfg.n_cores))])

                  # ------- Phase B1 (one-group lookahead: score path +
                  # one-hot for g+1 are emitted before the gather-dependent
                  # mult of g, so the in-order DVE queue never stalls on a
                  # ready-to-run op) -------
                  b1_groups = [(g0, min(GB, BPC - g0)) for g0 in range(0, BPC, GB)]
                  b1_jb = []
                  _jb = 0
                  for g0, gw in b1_groups:
                      b1_jb.append(_jb)
                      _jb += gw * T_B

                  with tc.tile_pool(name="b1_ps", bufs=3, space="PSUM") as psp, \
                       tc.tile_pool(name="b1_hp", bufs=2, space="PSUM") as hpp, \
                       tc.tile_pool(name="c_tp", bufs=1, space="PSUM") as ctp, \
                       tc.tile_pool(name="c_ps", bufs=2, space="PSUM") as cps:
                    def b1_pre(gi):
                        """mtall + score path (independent of gathers)."""
                        g0, gw = b1_groups[gi]
                        NJ = gw * T_B
                        jb = b1_jb[gi]
                        mtall = mtp.tile([P, GB * T_B, P], F16, tag="mt")
                        in0 = AP(iota[:].tensor, iota[:].offset,
                                 [list(iota[:].ap[0]), [0, NJ], [1, P]])
                        in1 = AP(drp[:].tensor, drp[:].offset + jb * 2,
                                 [list(drp[:].ap[0]), [2, NJ], [0, P // 2], [1, 2]])
                        nc.vector.tensor_tensor(out=mtall[:, 0:NJ, :], in0=in0,
                                                in1=in1, op=ALU.is_equal)
                        epl_v = epl[:, jb * H:(jb + NJ) * H].rearrange(
                            "p (j h) -> p j h", j=NJ)
                        lr = smp.tile([P, GB * T_B, H], F32, tag="lr")
                        nc.scalar.activation(out=lr[:, 0:NJ, :], in_=epl_v,
                                             func=AF.Prelu, alpha=NEG_SLOPE)
                        exd = smp.tile([P, GB * T_B, H, 2], F16, tag="exd")
                        for k in range(2):
                            od = AP(exd[:].tensor, exd[:].offset + k,
                                    [list(exd[:].ap[0]), [2 * H, NJ], [2, H],
                                     [1, 1]])
                            nc.scalar.activation(out=od, in_=lr[:, 0:NJ, :],
                                                 func=AF.Exp, bias=-1.0)
                        return mtall, exd

                    pre_live = {0: b1_pre(0)}
                    for gi, (g0, gw) in enumerate(b1_groups):
                        NJ = gw * T_B
                        jbase = b1_jb[gi]
                        sx = sxp.tile([P, GB * T_B, R1], F16, tag="sx")
                        nc.gpsimd.dma_gather(
                            out_ap=sx[:, 0:gw * T_LO, :],
                            in_ap=xp_tab[0:LO, :],
                            idxs_ap=idxlo[:, g0 * T_LO * 8:(g0 + gw) * T_LO * 8],
                            num_idxs=gw * T_LO * P, num_idxs_reg=gw * T_LO * P,
                            elem_size=R1, single_packet=False)
                        if T_HI:
                            nc.gpsimd.dma_gather(
                                out_ap=sx[:, gw * T_LO:NJ, :],
                                in_ap=xp_tab[LO:NPAD, :],
                                idxs_ap=idxhi[:, g0 * T_HI * 8:(g0 + gw) * T_HI * 8],
                                num_idxs=gw * T_HI * P, num_idxs_reg=gw * T_HI * P,
                                elem_size=R1, single_packet=False)
                        if gi + 1 < len(b1_groups):
                            pre_live[gi + 1] = b1_pre(gi + 1)
                        mtall, exd = pre_live.pop(gi)
                        rta = rhp.tile([P, GB * T_B, W1COLS], F16, tag="rta")
                        # exp column for denominator
                        nc.scalar.copy(
                            out=rta[:, 0:NJ, HC:HC + H],
                            in_=AP(exd[:].tensor, exd[:].offset,
                                   [list(exd[:].ap[0]), [2 * H, NJ], [2, H]]))
                        # messages: x_src * exp (pair-duplicated AP keeps
                        # 2x). Emitted per block so the PE can start a block's
                        # matmuls while the next block's messages multiply.
                        blk_ranges = []
                        for b in range(gw):
                            blk_ranges.append((b * T_LO, (b + 1) * T_LO))
                            blk_ranges.append((gw * T_LO + b * T_HI,
                                               gw * T_LO + (b + 1) * T_HI))
                        for (ja, jb_r) in blk_ranges:
                            nw = jb_r - ja
                            in1m = AP(exd[:].tensor, exd[:].offset + ja * 2 * H,
                                      [list(exd[:].ap[0]), [2 * H, nw], [2, H],
                                       [0, HID // 2], [1, 2]])
                            nc.vector.tensor_tensor(
                                out=rta[:, ja:jb_r, 0:HC].rearrange(
                                    "p j (h c) -> p j h c", h=H),
                                in0=sx[:, ja:jb_r, :].rearrange(
                                    "p j (h c) -> p j h c", h=H),
                                in1=in1m, op=ALU.mult)
                        ep = hwp.tile([P, GB * HC], F16, tag="ep")
                        rp = hwp.tile([P, GB * HC], F16, tag="rp")
                        for b in range(gw):
                            blk = g0 + b
                            psb = psp.tile([P, HC + H], F32, tag="psb")
                            tiles = ([b * T_LO + t for t in range(T_LO)]
                                     + [gw * T_LO + b * T_HI + t
                                        for t in range(T_HI)])
                            # self-loop term: exp_self (x) own features, added
                            # via one identity matmul into the accumulation
                            xself = sfp.tile([P, HC], F16, tag="xself")
                            nc.sync.dma_start(
                                out=xself[:],
                                in_=xp_own[blk * P:(blk + 1) * P, :])
                            st = sfp.tile([P, HC + H], F16, tag="st")
                            for h in range(H):
                                nc.scalar.activation(
                                    out=st[:, h * HID:(h + 1) * HID],
                                    in_=xself[:, h * HID:(h + 1) * HID],
                                    func=AF.Copy,
                                    scale=eself[:, blk * H + h:blk * H + h + 1])
                            nc.scalar.copy(
                                out=st[:, HC:HC + H],
                                in_=eself[:, blk * H:(blk + 1) * H])
                            for i, j in enumerate(tiles):
                                nc.tensor.matmul(
                                    out=psb[:], lhsT=mtall[:, j, 0:P],
                                    rhs=rta[:, j, 0:HC + H],
                                    start=(i == 0), stop=False)
                            nc.tensor.matmul(
                                out=psb[:], lhsT=idn16[:], rhs=st[:],
                                start=(len(tiles) == 0), stop=True)
                            # epilogue: h = ELU(psum/denom [+ b1])
                            rec = smp.tile([P, H], F32, tag="rec")
                            nc.vector.reciprocal(out=rec[:], in_=psb[:, HC:HC + H])
                            o_rp = rp[:, b * HC:(b + 1) * HC]
                            o_ep = ep[:, b * HC:(b + 1) * HC]
                            if cfg.has_b1:
                                hb = hwp.tile([P, HC], F32, tag="hb")
                                for h in range(H):
                                    nc.scalar.mul(out=hb[:, h * HID:(h + 1) * HID],
                                                  in_=psb[:, h * HID:(h + 1) * HID],
                                                  mul=rec[:, h:h + 1])
                                nc.vector.tensor_tensor(out=hb[:], in0=hb[:],
                                                        in1=b1t[:], op=ALU.add)
                                nc.scalar.activation(out=o_rp, in_=hb[:],
                                                     func=AF.Relu)
                                mn = hwp.tile([P, HC], F32, tag="mn")
                                nc.vector.tensor_scalar(
                                    out=mn[:], in0=hb[:], scalar1=0.0,
                                    scalar2=None, op0=ALU.min)
                                nc.scalar.activation(out=o_ep, in_=mn[:],
                                                     func=AF.Exp)
                            else:
                                # relu(x*rec)=relu(x)*rec; exp(min(x*rec,0)) =
                                # exp(-relu(-x*rec)) -- no DVE min needed
                                nrec = smp.tile([P, H], F32, tag="nrec")
                                nc.vector.tensor_scalar(
                                    out=nrec[:], in0=rec[:], scalar1=-1.0,
                                    scalar2=None, op0=ALU.mult)
                                mn = hwp.tile([P, HC], F16, tag="mn")
                                for h in range(H):
                                    nc.scalar.activation(
                                        out=o_rp[:, h * HID:(h + 1) * HID],
                                        in_=psb[:, h * HID:(h + 1) * HID],
                                        func=AF.Relu, scale=rec[:, h:h + 1])
                                    nc.scalar.activation(
                                        out=mn[:, h * HID:(h + 1) * HID],
                                        in_=psb[:, h * HID:(h + 1) * HID],
                                        func=AF.Relu, scale=nrec[:, h:h + 1])
                                nc.scalar.activation(
                                    out=o_ep, in_=mn[:], func=AF.Exp, scale=-1.0)
                        # h = relu_part + exp_part - 1: accumulate on the
                        # PE (identity stationary), then one Act copy w/ bias
                        hps = hpp.tile([P, GB * HC], F32, tag="hps")
                        nc.tensor.matmul(out=hps[:, 0:gw * HC], lhsT=idn16[:],
                                         rhs=ep[:, 0:gw * HC],
                                         start=True, stop=False)
                        nc.tensor.matmul(out=hps[:, 0:gw * HC], lhsT=idn16[:],
                                         rhs=rp[:, 0:gw * HC],
                                         start=False, stop=True)
                        nc.scalar.activation(
                            out=h_sb[:, g0 * HC:(g0 + gw) * HC],
                            in_=hps[:, 0:gw * HC], func=AF.Copy, bias=-1.0)
                        # ---- fused Phase C for this group's blocks ----
                        o2 = cop.tile([P, GB * R2], F16, tag="o2")
                        for b in range(gw):
                            blk = g0 + b
                            ob = b * R2
                            p2 = cps.tile([P, C2], F32, tag="p2")
                            for j in range(HC // P):
                                pt = ctp.tile([P, P], F16, tag="pt")
                                nc.tensor.transpose(
                                    out=pt[:],
                                    in_=h_sb[:, blk * HC + j * P:
                                             blk * HC + (j + 1) * P],
                                    identity=idn16[:])
                                hT = chp.tile([P, P], F16, tag="hT")
                                nc.scalar.copy(out=hT[:], in_=pt[:])
                                nc.tensor.matmul(
                                    out=p2[:], lhsT=hT[:],
                                    rhs=w2s[:, j * C2:(j + 1) * C2],
                                    start=(j == 0), stop=(j == HC // P - 1))
                            # row: [feats | as_hi | as_lo | 0pad]; the o2
                            # ring buffers keep their pad zeros after the
                            # first cycle, so only zero the first 3 groups
                            if gi < 3:
                                nc.vector.memset(o2[:, ob + OC + 2:ob + R2], 0.0)
                            nc.scalar.copy(out=o2[:, ob:ob + OC + 1],
                                           in_=p2[:, 0:OC + 1])
                            alo = cop.tile([P, 1], F32, tag="alo")
                            nc.vector.tensor_tensor(
                                out=alo[:], in0=p2[:, OC:OC + 1],
                                in1=o2[:, ob + OC:ob + OC + 1],
                                op=ALU.subtract)
                            nc.vector.tensor_copy(
                                out=o2[:, ob + OC + 1:ob + OC + 2], in_=alo[:])
                            nc.scalar.copy(out=al2d[:, 2 * blk:2 * blk + 1],
                                           in_=p2[:, OC + 1:OC + 2])
                            ado = cop.tile([P, 1], F32, tag="ado")
                            nc.vector.tensor_tensor(
                                out=ado[:], in0=p2[:, OC + 1:OC + 2],
                                in1=al2d[:, 2 * blk:2 * blk + 1],
                                op=ALU.subtract)
                            nc.vector.tensor_copy(
                                out=al2d[:, 2 * blk + 1:2 * blk + 2], in_=ado[:])
                            nc.scalar.copy(
                                out=al2s[:, 2 * blk:2 * blk + 2],
                                in_=o2[:, ob + OC:ob + OC + 2])
                        cbase = xp2_own[g0 * P:(g0 + gw) * P, :]
                        codram = AP(cbase.tensor, cbase.offset,
                                    [[R2, P], [P * R2, gw], [1, R2]])
                        nc.sync.dma_start(out=codram, in_=o2[:, 0:gw * R2])

                # ------- sad pre-pass + B2 (shared SBUF pool context) -------
                sad_all = consts.tile([P, NJ_ALL], F32)
                with tc.tile_pool(name="b2_sf", bufs=2) as sfp2, \
                     tc.tile_pool(name="sp_m2", bufs=3) as sm2p, \
                     tc.tile_pool(name="sp_mt", bufs=2) as smtp, \
                     tc.tile_pool(name="b2_sx", bufs=6) as sxp2, \
                     tc.tile_pool(name="b2_mt", bufs=4) as mtp2, \
                     tc.tile_pool(name="b2_rhs", bufs=3) as rhp2, \
                     tc.tile_pool(name="b2_sm", bufs=3) as smp2, \
                     tc.tile_pool(name="b2_z", bufs=3) as zp:
                  if os.environ.get("KNOAG"):
                    nc.gpsimd.dma_start(out=xp2_tab[0:NPC, :], in_=xp2_own[:, :])
                  else:
                    nc.gpsimd.collective_compute(
                        "AllGather", mybir.AluOpType.bypass,
                        ins=[xp2_own.opt()],
                        outs=[xp2_tab.opt()],
                        replica_groups=[list(range(cfg.n_cores))])

                  # ----- B2 with software-pipelined dst-score (sad) -----
                  # Iteration g: issue gathers(g); build mtall(g+1) and its
                  # sad (PE transpose of the one-hot + tiny matmuls vs al2d);
                  # consume mtall(g)/sad_all(g) for scores + aggregation.
                  groups = [(g0, min(GB, BPC - g0)) for g0 in range(0, BPC, GB)]
                  jbases = []
                  _jb = 0
                  for g0, gw in groups:
                      jbases.append(_jb)
                      _jb += gw * T_B

                  with tc.tile_pool(name="sp_tp", bufs=2, space="PSUM") as stpp, \
                       tc.tile_pool(name="sp_sp", bufs=2, space="PSUM") as sspp, \
                       tc.tile_pool(name="b2_ps", bufs=4, space="PSUM") as psp2:

                    def build_mtall(gi):
                        g0, gw = groups[gi]
                        NJ = gw * T_B
                        jb = jbases[gi]
                        mtall = mtp2.tile([P, GB * T_B, P], F16, tag="mt2")
                        in0 = AP(iota[:].tensor, iota[:].offset,
                                 [list(iota[:].ap[0]), [0, NJ], [1, P]])
                        in1 = AP(drp[:].tensor, drp[:].offset + jb * 2,
                                 [list(drp[:].ap[0]), [2, NJ], [0, P // 2], [1, 2]])
                        nc.vector.tensor_tensor(out=mtall[:, 0:NJ, :], in0=in0,
                                                in1=in1, op=ALU.is_equal)
                        return mtall

                    def build_sad(gi, mtall):
                        g0, gw = groups[gi]
                        NJ = gw * T_B
                        jb = jbases[gi]
                        sadps = sspp.tile([P, GB * T_B, 2], F32, tag="sadps")
                        TPB = 8
                        for k0 in range(0, NJ, TPB):
                            kw = min(TPB, NJ - k0)
                            mps = stpp.tile([P, TPB, P], F16, tag="mps")
                            for k in range(kw):
                                nc.tensor.transpose(out=mps[:, k, :],
                                                    in_=mtall[:, k0 + k, :],
                                                    identity=idn16[:])
                            m2 = sm2p.tile([P, TPB, P], F16, tag="m2sb")
                            if (k0 // TPB) % 2 == 0:
                                nc.vector.tensor_copy(out=m2[:, 0:kw, :],
                                                      in_=mps[:, 0:kw, :])
                            else:
                                nc.scalar.copy(out=m2[:, 0:kw, :],
                                               in_=mps[:, 0:kw, :])
                            for k in range(kw):
                                j = k0 + k
                                blk = g0 + (j // T_LO if j < gw * T_LO
                                            else (j - gw * T_LO) // T_HI)
                                nc.tensor.matmul(
                                    out=sadps[:, j, :], lhsT=m2[:, k, :],
                                    rhs=al2d[:, 2 * blk:2 * blk + 2],
                                    start=True, stop=True)
                        sadsb = sm2p.tile([P, GB * T_B, 2], F32, tag="sadsb")
                        nc.scalar.copy(out=sadsb[:, 0:NJ, :],
                                       in_=sadps[:, 0:NJ, :])
                        nc.vector.tensor_tensor(
                            out=sad_all[:, jb:jb + NJ],
                            in0=AP(sadsb[:].tensor, sadsb[:].offset,
                                   [list(sadsb[:].ap[0]), [2, NJ]]),
                            in1=AP(sadsb[:].tensor, sadsb[:].offset + 1,
                                   [list(sadsb[:].ap[0]), [2, NJ]]),
                            op=ALU.add)

                    mt_live = {0: build_mtall(0)}
                    build_sad(0, mt_live[0])
                    if len(groups) > 1:
                        mt_live[1] = build_mtall(1)
                        build_sad(1, mt_live[1])
                    for gi, (g0, gw) in enumerate(groups):
                        NJ = gw * T_B
                        jbase = jbases[gi]
                        sx = sxp2.tile([P, GB * T_B, R2], F16, tag="sx2")
                        nc.gpsimd.dma_gather(
                            out_ap=sx[:, 0:gw * T_LO, :],
                            in_ap=xp2_tab[0:LO, :],
                            idxs_ap=idxlo[:, g0 * T_LO * 8:(g0 + gw) * T_LO * 8],
                            num_idxs=gw * T_LO * P, num_idxs_reg=gw * T_LO * P,
                            elem_size=R2, single_packet=False)
                        if T_HI:
                            nc.gpsimd.dma_gather(
                                out_ap=sx[:, gw * T_LO:NJ, :],
                                in_ap=xp2_tab[LO:NPAD, :],
                                idxs_ap=idxhi[:, g0 * T_HI * 8:(g0 + gw) * T_HI * 8],
                                num_idxs=gw * T_HI * P, num_idxs_reg=gw * T_HI * P,
                                elem_size=R2, single_packet=False)
                        mtall = mt_live.pop(gi)
                        # scores: (as_hi + as_lo) + sad_all, clamp
                        def col(tile_ap, c):
                            return AP(tile_ap.tensor, tile_ap.offset + c,
                                      [list(tile_ap.ap[0]), [R2, NJ]])
                        zal = smp2.tile([P, GB * T_B], F32, tag="zal")
                        nc.vector.tensor_tensor(out=zal[:, 0:NJ],
                                                in0=col(sx[:], OC),
                                                in1=col(sx[:], OC + 1),
                                                op=ALU.add)
                        ecl2 = smp2.tile([P, GB * T_B], F32, tag="ecl2")
                        nc.vector.tensor_tensor(
                            out=ecl2[:, 0:NJ], in0=zal[:, 0:NJ],
                            in1=sad_all[:, jbase:jbase + NJ], op=ALU.add)
                        nc.vector.tensor_scalar(
                            out=ecl2[:, 0:NJ], in0=ecl2[:, 0:NJ],
                            scalar1=EXP_CLAMP, scalar2=None, op0=ALU.min)
                        lr2 = smp2.tile([P, GB * T_B], F32, tag="lr2")
                        nc.scalar.activation(out=lr2[:, 0:NJ], in_=ecl2[:, 0:NJ],
                                             func=AF.Prelu, alpha=NEG_SLOPE)
                        exd2 = smp2.tile([P, GB * T_B, 2], F16, tag="exd2")
                        for k in range(2):
                            od = AP(exd2[:].tensor, exd2[:].offset + k,
                                    [list(exd2[:].ap[0]), [2, NJ], [1, 1]])
                            nc.scalar.activation(out=od, in_=lr2[:, 0:NJ],
                                                 func=AF.Exp, bias=-1.0)
                        rta = rhp2.tile([P, GB * T_B, OC + 1], F16, tag="rta2")
                        nc.scalar.copy(
                            out=rta[:, 0:NJ, OC],
                            in_=AP(exd2[:].tensor, exd2[:].offset,
                                   [list(exd2[:].ap[0]), [2, NJ]]))
                        in1m = AP(exd2[:].tensor, exd2[:].offset,
                                  [list(exd2[:].ap[0]), [2, NJ],
                                   [0, OC // 2], [1, 2]])
                        nc.vector.tensor_tensor(
                            out=rta[:, 0:NJ, 0:OC], in0=sx[:, 0:NJ, 0:OC],
                            in1=in1m, op=ALU.mult)
                        for b in range(gw):
                            blk = g0 + b
                            psb = psp2.tile([P, OC + 1], F32, tag="psb2")
                            tiles = ([b * T_LO + t for t in range(T_LO)]
                                     + [gw * T_LO + b * T_HI + t
                                        for t in range(T_HI)])
                            # self-loop term for layer 2 (scores on device)
                            e2p = sfp2.tile([P, 2], F32, tag="e2p")
                            nc.vector.tensor_tensor(
                                out=e2p[:], in0=al2s[:, 2 * blk:2 * blk + 2],
                                in1=al2d[:, 2 * blk:2 * blk + 2], op=ALU.add)
                            e2 = sfp2.tile([P, 1], F32, tag="e2")
                            nc.vector.tensor_tensor(
                                out=e2[:], in0=e2p[:, 0:1], in1=e2p[:, 1:2],
                                op=ALU.add)
                            nc.vector.tensor_scalar(
                                out=e2[:], in0=e2[:], scalar1=EXP_CLAMP,
                                scalar2=None, op0=ALU.min)
                            nc.scalar.activation(out=e2[:], in_=e2[:],
                                                 func=AF.Prelu, alpha=NEG_SLOPE)
                            x2f = sfp2.tile([P, 1], F32, tag="x2f")
                            nc.scalar.activation(out=x2f[:], in_=e2[:],
                                                 func=AF.Exp, bias=-1.0)
                            hfet = sfp2.tile([P, OC], F16, tag="hfet")
                            nc.sync.dma_start(
                                out=hfet[:],
                                in_=xp2_own[blk * P:(blk + 1) * P, 0:OC])
                            st2 = sfp2.tile([P, OC + 1], F16, tag="st2")
                            nc.scalar.activation(out=st2[:, 0:OC], in_=hfet[:],
                                                 func=AF.Copy, scale=x2f[:, 0:1])
                            nc.scalar.copy(out=st2[:, OC:OC + 1], in_=x2f[:])
                            for i, j in enumerate(tiles):
                                nc.tensor.matmul(
                                    out=psb[:], lhsT=mtall[:, j, 0:P],
                                    rhs=rta[:, j, 0:OC + 1],
                                    start=(i == 0), stop=False)
                            nc.tensor.matmul(
                                out=psb[:], lhsT=idn16[:], rhs=st2[:],
                                start=(len(tiles) == 0), stop=True)
                            rec = smp2.tile([P, 1], F32, tag="rec2")
                            nc.vector.reciprocal(out=rec[:], in_=psb[:, OC:OC + 1])
                            zb = zp.tile([P, OC], F32, tag="zb")
                            nc.scalar.activation(out=zb[:], in_=psb[:, 0:OC],
                                                 func=AF.Copy, scale=rec[:, 0:1])
                            if cfg.has_b2:
                                nc.vector.tensor_tensor(out=zb[:], in0=zb[:],
                                                        in1=b2t[:], op=ALU.add)
                            nc.sync.dma_start(out=z_t.ap()[blk * P:(blk + 1) * P, :],
                                              in_=zb[:])
                        if gi + 2 < len(groups):
                            mt_live[gi + 2] = build_mtall(gi + 2)
                            build_sad(gi + 2, mt_live[gi + 2])

    nc.compile()
    return nc


_CACHE = {}


def _get_built(cfg):
    key = (cfg, os.environ.get("KNOAG"), os.environ.get("KSHARED"))
    if key not in _CACHE:
        _CACHE[key] = build(cfg)
    return _CACHE[key]


class Runner:
    """Executes the compiled Bass module via PJRT/shard_map with inputs
    pre-sharded per device (no on-device resharding programs)."""

    def __init__(self, nc, n_cores):
        import jax
        from jax.sharding import Mesh, PartitionSpec, NamedSharding
        from jax.experimental.shard_map import shard_map
        from concourse import bass2jax

        bass2jax.install_neuronx_cc_hook()
        self.jax = jax
        self.nc = nc
        self.n_cores = n_cores

        pname = nc.partition_id_tensor.name if nc.partition_id_tensor else None
        in_names, out_names, out_avals = [], [], []
        for alloc in nc.m.functions[0].allocations:
            if not isinstance(alloc, mybir.MemoryLocationSet):
                continue
            name = alloc.memorylocations[0].name
            if alloc.kind == "ExternalInput":
                if name != pname:
                    in_names.append(name)
            elif alloc.kind == "ExternalOutput":
                out_names.append(name)
                out_avals.append(jax.core.ShapedArray(
                    tuple(alloc.tensor_shape), mybir.dt.np(alloc.dtype)))
        self.in_names, self.out_names, self.out_avals = in_names, out_names, out_avals
        all_in = list(in_names) + list(out_names)
        if pname is not None:
            all_in.append(pname)

        def _body(*args):
            operands = list(args)
            if pname is not None:
                operands.append(bass2jax.partition_id_tensor())
            outs = bass2jax._bass_exec_p.bind(
                *operands,
                out_avals=tuple(out_avals),
                in_names=tuple(all_in),
                out_names=tuple(out_names),
                lowering_input_output_aliases=(),
                sim_require_finite=True,
                sim_require_nnan=True,
                nc=nc,
            )
            return tuple(outs)

        self.devices = jax.devices()[:n_cores]
        self.mesh = Mesh(np.asarray(self.devices), ("core",))
        self.sh = NamedSharding(self.mesh, PartitionSpec("core"))
        nspec = (PartitionSpec("core"),)
        self.fn = jax.jit(
            shard_map(_body, mesh=self.mesh,
                      in_specs=nspec * (len(in_names) + len(out_names)),
                      out_specs=nspec * len(out_names), check_rep=False),
            keep_unused=True)
        self.dev_args = None

    def _shard(self, per_core):
        jax = self.jax
        a0 = np.asarray(per_core[0])
        gshape = (self.n_cores * a0.shape[0],) + a0.shape[1:]
        bufs = [jax.device_put(np.asarray(per_core[c]), self.devices[c])
                for c in range(self.n_cores)]
        return jax.make_array_from_single_device_arrays(gshape, self.sh, bufs)

    def set_inputs(self, in_maps):
        args = [self._shard([m[name] for m in in_maps])
                for name in self.in_names]
        for av in self.out_avals:
            z = np.zeros(av.shape, av.dtype)
            args.append(self._shard([z] * self.n_cores))
        self.dev_args = args

    def call(self):
        outs = self.fn(*self.dev_args)
        self.jax.block_until_ready(outs)
        return outs

    def bench(self, k_hi=110, k_lo=10, reps=5):
        """Marginal per-exec time via async-pipelined dispatch: issue k
        back-to-back calls of the single-exec jitted fn, block at the end."""
        import time

        def run_k(k):
            out = None
            for _ in range(k):
                out = self.fn(*self.dev_args)
            self.jax.block_until_ready(out)

        run_k(3)  # warm
        t_lo, t_hi = [], []
        for _ in range(reps):
            t0 = time.perf_counter()
            run_k(k_lo)
            t_lo.append(time.perf_counter() - t0)
            t0 = time.perf_counter()
            run_k(k_hi)
            t_hi.append(time.perf_counter() - t0)
        per_iter = (min(t_hi) - min(t_lo)) / (k_hi - k_lo)
        return per_iter, min(t_lo), min(t_hi)

    def run(self, in_maps):
        self.set_inputs(in_maps)
        outs = self.call()
        res = []
        for c in range(self.n_cores):
            d = {}
            for i, name in enumerate(self.out_names):
                g = np.asarray(outs[i])
                n0 = self.out_avals[i].shape[0]
                d[name] = g.reshape(self.n_cores, n0, *self.out_avals[i].shape[1:])[c]
            res.append(d)
        return res


_RUNNERS = {}


def _get_runner(cfg, nc):
    key = id(nc)
    if key not in _RUNNERS:
        _RUNNERS[key] = Runner(nc, cfg.n_cores)
    return _RUNNERS[key]


def kernel(x, edge_index, W1, a1_src, a1_dst, b1, W2, a2_src, a2_dst, b2):
    x = np.asarray(x)
    cfg, in_maps, pid_of = prep(x, edge_index, W1, a1_src, a1_dst, b1,
                                W2, a2_src, a2_dst, b2)
    nc = _get_built(cfg)
    runner = _get_runner(cfg, nc)
    results = runner.run(in_maps)
    z_full = np.concatenate([results[c]["z"] for c in range(cfg.n_cores)],
                            axis=0)
    return np.ascontiguousarray(z_full[pid_of]).astype(np.float32)
